# revision 80
# baseline (speedup 1.0000x reference)
"""Trainium2 Bass kernel for LocalSparseAttention.

Problem (hardcoded): B=2, S=2048, D=1024, H=16, HD=64, WINDOW=128 (band
|i-j| <= 64), fp32 I/O.

Sharding: 8 cores = 2 batches x 4 head-groups (4 heads each). Each core:
  - qk projection into transposed layout [512, 2048] (head-pair packed)
  - v projection into natural layout, 17 (possibly 64-shifted) seq chunks
    (boundary tiles reuse shifted chunks; masks dedup the overlap)
  - banded attention per 128-query tile with a 256-key window:
      scores  -> exp on ACT -> 0/1 band-mask multiply on DVE
      AV^T    : lhsT=exp tile [keys, q], rhs=v (+ ones col) -> PSUM [q, 65]
                (full 128x128 PE efficiency; denominator = ones column)
      norm    : per-partition reciprocal + stride-0-broadcast multiply
                (DVE for even head, GpSimd for odd head)
      PE transpose of normalized [q, 64] tiles back into aoT [dims, seq]
  - output projection -> fp16 partial [2048, 1024]
Host: fp16 casts/transposes in, sum of 4 partials per batch + fused bias
(b_out + b_v @ w_out) out.

All matmuls run in fp16 (1 cycle/row on PE) with fp32 PSUM accumulation;
softmax exp input stays fp32.
"""
import sys

if "/opt/trn_rl_repo" not in sys.path:
    sys.path.insert(0, "/opt/trn_rl_repo")

import numpy as np

import concourse.bass as bass
import concourse.mybir as mybir
import concourse.tile as tile
from concourse import bacc
from concourse.bass import AP
from concourse.bass_utils import run_bass_kernel_spmd

B, S, D, H, HD = 2, 2048, 1024, 16, 64
SCALE = HD**-0.5
C_SUB = 4.0  # subtracted from all scores via the exp bias; cancels in softmax

F16 = mybir.dt.float16
F32 = mybir.dt.float32

# 19 key/value chunk offsets: 15 shifted (128c+64) + aligned 0,128,1792,1920
OFFS = [128 * c + 64 for c in range(15)] + [0, 128, 1792, 1920]

N_WARMUP = 20


def _chunk_pair(i):
    # Boundary tiles reuse shifted chunks (the mask dedups the overlap),
    # so only 17 v chunks are ever materialized (15 shifted + 0 + 1920).
    if i == 0:
        return 15, 0
    if i == 15:
        return 14, 18
    return i - 1, i


def _half_mask(q_base, k_base, own_lo, own_hi):
    kp = np.arange(128)[:, None]
    q = np.arange(128)[None, :]
    k = k_base + kp
    valid = (k >= own_lo) & (k < own_hi) & (np.abs((q_base + q) - k) <= 64)
    return valid.astype(np.float16)


def _build_pair_masks():
    # variant 0: (tile 0, interior) — c4=0 pair 0
    # variant 1: (interior, interior)
    # variant 2: (interior, tile 15) — c4=3 pair 1
    # Each [kp, q] mask owns a disjoint global-key range so overlapping
    # chunk halves never double-count a key.
    mp = np.zeros((128, 3, 2, 2, 128), np.float16)
    int0 = _half_mask(128, 64, 64, 192)       # any interior tile, half 0
    int1 = _half_mask(128, 192, 192, 320)     # any interior tile, half 1
    mp[:, 1, :, 0] = int0[:, None]
    mp[:, 1, :, 1] = int1[:, None]
    mp[:, 0] = mp[:, 2] = mp[:, 1]
    mp[:, 0, 0, 0] = _half_mask(0, 0, 0, 128)      # tile 0 vs chunk 15
    mp[:, 0, 0, 1] = _half_mask(0, 64, 128, 192)   # tile 0 vs chunk 0
    mp[:, 2, 1, 0] = _half_mask(1920, 1856, 1856, 1984)  # tile 15 / chunk 14
    mp[:, 2, 1, 1] = _half_mask(1920, 1920, 1984, 2048)  # tile 15 / chunk 18
    return mp


def _bcast_free(ap, n):
    # append a stride-0 free dim of size n (broadcast along free axis)
    return AP(ap.tensor, ap.offset, list(ap.ap) + [[0, n]])


def _build_program():
    nc = bacc.Bacc("TRN2", debug=False, num_devices=8)

    xT_d = nc.dram_tensor("xT", [D, S], F16, kind="ExternalInput").ap()
    wqk_d = nc.dram_tensor("wqk", [D, 512], F16, kind="ExternalInput").ap()
    wv_d = nc.dram_tensor("wv", [D, 256], F16, kind="ExternalInput").ap()
    wout_d = nc.dram_tensor("wout", [256, D], F16, kind="ExternalInput").ap()
    bqk_d = nc.dram_tensor("bqk", [128, 4], F32, kind="ExternalInput").ap()
    masks_d = nc.dram_tensor("masks", [128, 3, 2, 2, 128], F16,
                             kind="ExternalInput").ap()
    ident_d = nc.dram_tensor("ident", [128, 128], F16,
                             kind="ExternalInput").ap()
    out_d = nc.dram_tensor("out", [S, D], F16, kind="ExternalOutput").ap()

    with tile.TileContext(nc) as tc:
        with (
            tc.tile_pool(name="const", bufs=1) as cpool,
            tc.tile_pool(name="work", bufs=2) as wpool,
            tc.tile_pool(name="expp", bufs=10) as epool,
            tc.tile_pool(name="ysb", bufs=8) as ypool,
            tc.tile_pool(name="ps512", bufs=2, space="PSUM") as ps512,
            tc.tile_pool(name="psv", bufs=2, space="PSUM") as psv,
            tc.tile_pool(name="pssc", bufs=2, space="PSUM") as pssc,
            tc.tile_pool(name="psavT", bufs=2, space="PSUM") as psavT,
        ):
            # ---- persistent SBUF tensors ----
            xT_sb = cpool.tile([128, 8, S], F16, tag="xT")
            wqk_sb = cpool.tile([128, 8, 512], F16, tag="wqk")
            wv_sb = cpool.tile([128, 8, 256], F16, tag="wv")
            wout_sb = cpool.tile([128, 2, D], F16, tag="wout")
            bqk_sb = cpool.tile([128, 4], F32, tag="bqk")
            masks_sb = cpool.tile([128, 3, 2, 2, 128], F16, tag="masks")
            ident_sb = cpool.tile([128, 128], F16, tag="ident")
            qk_sb = cpool.tile([128, 4, S], F16, tag="qk")
            v_sb = cpool.tile([128, 19, 4, 65], F16, tag="v")
            aoT_sb = cpool.tile([128, 2, S], F16, tag="aoT")
            negc_sb = cpool.tile([128, 1], F32, tag="negc")

            # ---- input DMAs: ~256KB chunks in PE-consumption order, issue
            # cost (~650ns each) split across the two HWDGE queues (sync +
            # scalar) so transfers start early and overlap across rings ----
            xT_r = xT_d.rearrange("(ko kp) s -> kp ko s", kp=128)
            wqk_r = wqk_d.rearrange("(ko kp) n -> kp ko n", kp=128)
            wv_r = wv_d.rearrange("(ko kp) n -> kp ko n", kp=128)
            wout_r = wout_d.rearrange("(t p) n -> p t n", p=128)
            # Fine-grained DMAs on the sync queue in strict consumption
            # order: small chunks land with low latency and the issue stream
            # itself paces transfers so later chunks never steal bandwidth.
            # first 16 issues alternate sync/scalar: scalar is idle until
            # ~25us (qk drains live on DVE now), and two issue queues get
            # all 8 DMA rings transferring ~2.5us sooner
            for kt in range(8):
                eng_a = nc.sync if kt % 2 == 0 else nc.scalar
                eng_b = nc.scalar if kt % 2 == 0 else nc.sync
                eng_a.dma_start(out=wqk_sb[:, kt], in_=wqk_r[:, kt])
                eng_b.dma_start(out=xT_sb[:, kt, 0:512],
                                in_=xT_r[:, kt, 0:512])
            nc.sync.dma_start(out=bqk_sb[:], in_=bqk_d)
            nc.sync.dma_start(out=masks_sb[:], in_=masks_d)
            for kt in range(8):
                nc.sync.dma_start(out=xT_sb[:, kt, 512:1024],
                                  in_=xT_r[:, kt, 512:1024])
            for kt in range(8):
                nc.sync.dma_start(out=wv_sb[:, kt], in_=wv_r[:, kt])
            nc.sync.dma_start(out=ident_sb[:], in_=ident_d)
            nc.sync.dma_start(out=wout_sb[:], in_=wout_r[:])
            for kt in range(8):
                nc.sync.dma_start(out=xT_sb[:, kt, 1024:2048],
                                  in_=xT_r[:, kt, 1024:2048])

            # ---- constants (wsrc first so the PE warmup can start ASAP;
            # negc / v-ones deferred past the prologue so DVE drains the
            # first qk PSUM groups without queueing behind them) ----
            wsrc = cpool.tile([128, 512], F16, tag="wsrc")
            nc.vector.memset(wsrc[:, 0:128], 0.0)
            nc.vector.memset(wsrc[:, 128:512], 0.0)

            # ---- PE warmup: dummy matmuls on zeroed SBUF so the HAM
            # clock-gate ramps while the first input DMAs land ----
            wdst = cpool.tile([128, 512], F16, tag="wdst")
            wps = ps512.tile([128, 512], F32, tag="ps512")
            for w in range(N_WARMUP):
                nc.tensor.matmul(
                    out=wps[:],
                    lhsT=wsrc[:, 0:128],
                    rhs=wsrc[:],
                    start=(w == 0),
                    stop=(w == N_WARMUP - 1),
                )
            nc.scalar.copy(out=wdst[:], in_=wps[:])

            # ---- emission helpers ----
            def emit_qk_chunk(ns, pools=None, on_act=False):
                # all 4 m-tiles of q/k projection for seq chunk ns.
                # on_act: drain PSUM via ACT (idle in the prologue) instead
                # of DVE (busy mid-kernel).
                for m in range(4):
                    scale = SCALE if m < 2 else 1.0
                    if pools is None:
                        ps = ps512.tile([128, 512], F32, tag="ps512")
                    else:
                        pool, tg = pools[m % len(pools)]
                        ps = pool.tile([128, 512], F32, tag=tg)
                    for kt in range(8):
                        nc.tensor.matmul(
                            out=ps[:],
                            lhsT=wqk_sb[:, kt, m * 128:(m + 1) * 128],
                            rhs=xT_sb[:, kt, ns * 512:(ns + 1) * 512],
                            start=(kt == 0),
                            stop=(kt == 7),
                        )
                    if on_act:
                        nc.scalar.activation(
                            out=qk_sb[:, m, ns * 512:(ns + 1) * 512],
                            in_=ps[:],
                            func=mybir.ActivationFunctionType.Identity,
                            bias=bqk_sb[:, m:m + 1],
                            scale=scale,
                        )
                    else:
                        nc.vector.tensor_scalar(
                            out=qk_sb[:, m, ns * 512:(ns + 1) * 512],
                            in0=ps[:],
                            scalar1=scale,
                            scalar2=bqk_sb[:, m:m + 1],
                            op0=mybir.AluOpType.mult,
                            op1=mybir.AluOpType.add,
                        )

            def emit_qk_chunk_ktmajor(ns, pools):
                # kt-major: 4 open PSUM groups, consuming each xT k-chunk
                # as its DMA lands (used for the DMA-paced prologue chunks)
                pss = []
                for m in range(4):
                    pool, tg = pools[m % len(pools)]
                    ps = pool.tile([128, 512], F32, tag=tg, name=f"qkm{m}")
                    pss.append(ps)
                for kt in range(8):
                    for m in range(4):
                        nc.tensor.matmul(
                            out=pss[m][:],
                            lhsT=wqk_sb[:, kt, m * 128:(m + 1) * 128],
                            rhs=xT_sb[:, kt, ns * 512:(ns + 1) * 512],
                            start=(kt == 0),
                            stop=(kt == 7),
                        )
                for m in range(4):
                    nc.scalar.activation(
                        out=qk_sb[:, m, ns * 512:(ns + 1) * 512],
                        in_=pss[m][:],
                        func=mybir.ActivationFunctionType.Identity,
                        bias=bqk_sb[:, m:m + 1],
                        scale=SCALE if m < 2 else 1.0,
                    )

            def emit_v_chunk(c):
                off = OFFS[c]
                ps = psv.tile([128, 256], F32, tag="psv")
                for kt in range(8):
                    nc.tensor.matmul(
                        out=ps[:],
                        lhsT=xT_sb[:, kt, off:off + 128],
                        rhs=wv_sb[:, kt, :],
                        start=(kt == 0),
                        stop=(kt == 7),
                    )
                nc.scalar.copy(
                    out=v_sb[:, c, :, 0:64],
                    in_=ps[:].rearrange("p (h d) -> p h d", h=4),
                )

            def emit_v_pair(c):
                # two adjacent chunks through one PSUM bank + one ACT copy
                ps = psv.tile([128, 2, 256], F32, tag="psv", name="psvp")
                for j in range(2):
                    off = OFFS[c + j]
                    for kt in range(8):
                        nc.tensor.matmul(
                            out=ps[:, j, :],
                            lhsT=xT_sb[:, kt, off:off + 128],
                            rhs=wv_sb[:, kt, :],
                            start=(j == 0 and kt == 0),
                            stop=(j == 1 and kt == 7),
                        )
                nc.scalar.copy(
                    out=v_sb[:, c:c + 2, :, 0:64],
                    in_=ps[:].rearrange("p c (h d) -> p c h d", h=4),
                )

            def emit_scores(c4, hp):
                # scores + exp for both heads of the pair
                ex_big0 = epool.tile([128, 4, 2, 128], F16, tag="exp")
                ex_big1 = epool.tile([128, 4, 2, 128], F16, tag="exp")
                ex_big = {0: ex_big0, 1: ex_big1}
                for pair in range(2):
                    if c4 == 0 and pair == 0:
                        pv = 0
                    elif c4 == 3 and pair == 1:
                        pv = 2
                    else:
                        pv = 1
                    sc_h0 = pssc.tile([128, 2, 2, 128], F32, tag="pssc")
                    sc_h1 = pssc.tile([128, 2, 2, 128], F32, tag="pssc")
                    scs = {0: sc_h0, 1: sc_h1}
                    for iw in range(2):
                        ii = pair * 2 + iw
                        i = c4 * 4 + ii
                        cA, cB = _chunk_pair(i)
                        for hh in range(2):
                            po = hh * 64
                            for half, cc in enumerate((cA, cB)):
                                off = OFFS[cc]
                                nc.tensor.matmul(
                                    out=scs[hh][:, iw, half, :],
                                    lhsT=qk_sb[po:po + 64, 2 + hp,
                                               off:off + 128],
                                    rhs=qk_sb[po:po + 64, hp,
                                              i * 128:(i + 1) * 128],
                                    start=(iw == 0 and half == 0),
                                    stop=(iw == 1 and half == 1),
                                )
                    for hh in range(2):
                        # exp(score - C) on ACT (one op per ii-pair),
                        # band-zeroing via 0/1 mask multiply on DVE
                        sl = slice(pair * 2, pair * 2 + 2)
                        nc.scalar.activation(
                            out=ex_big[hh][:, sl],
                            in_=scs[hh][:],
                            func=mybir.ActivationFunctionType.Exp,
                            bias=negc_sb[:],
                        )
                        nc.vector.tensor_mul(
                            out=ex_big[hh][:, sl],
                            in0=ex_big[hh][:, sl],
                            in1=masks_sb[:, pv],
                        )
                return ex_big

            def emit_avT(c4, hp, ex_big):
                # AV^T: [q, dims+1] PSUM per (ii, head); ones column of v
                # yields the softmax denominator in col 64.
                avts = {}
                for hh in range(2):
                    h = 2 * hp + hh
                    avt = psavT.tile([128, 4, 65], F32, tag="psavT",
                                     padded_shape=[128, 4, 128])
                    for ii in range(4):
                        cA, cB = _chunk_pair(c4 * 4 + ii)
                        for half, cc in enumerate((cA, cB)):
                            nc.tensor.matmul(
                                out=avt[:, ii, :],
                                lhsT=ex_big[hh][:, ii, half, :],
                                rhs=v_sb[:, cc, h, 0:65],
                                start=(ii == 0 and half == 0),
                                stop=(ii == 3 and half == 1),
                            )
                    avts[hh] = avt
                # per-query reciprocal of the denominators, then normalize
                # with a stride-0 broadcast multiply (q is on partitions)
                recip = wpool.tile([128, 2, 4], F32, tag="recip")
                # ii-major layout so each ii slice is a contiguous [128, 128]
                # holding both heads — one PE transpose covers the pair
                avn = wpool.tile([128, 4, 2, 64], F16, tag="avn")
                for hh in range(2):
                    nc.vector.reciprocal(
                        out=recip[:, hh, :],
                        in_=avts[hh][:, :, 64:65].rearrange("p a b -> p (a b)"),
                    )
                    nc.vector.tensor_mul(
                        out=avn[:, :, hh, :],
                        in0=avts[hh][:, :, 0:64],
                        in1=_bcast_free(recip[:, hh, :], 64),
                    )
                return avn

            def emit_transposes(c4, hp, avn, split_copy=False):
                # PE-transpose normalized [q, 64] tiles into aoT [dims, seq]
                psT = psv.tile([128, 4, 128], F16, tag="psv")
                for ii in range(4):
                    nc.tensor.transpose(
                        out=psT[:, ii, :],
                        in_=avn[:, ii].rearrange("p a b -> p (a b)"),
                        identity=ident_sb[:],
                    )
                sl0 = c4 * 512
                if not split_copy:
                    # DVE: its queue clears sooner than ACT's (which is
                    # backed up behind the next block's exp ops)
                    nc.vector.tensor_copy(
                        out=aoT_sb[:, hp, sl0:sl0 + 512].rearrange(
                            "p (a b) -> p a b", a=4),
                        in_=psT[:],
                    )
                else:
                    # final block: per-ii copies, alternating ACT/DVE so
                    # the last outproj tiles' inputs land two at a time
                    for ii in range(4):
                        dst = aoT_sb[:, hp, sl0 + ii * 128:sl0 + ii * 128 + 128]
                        if ii % 2 == 0:
                            nc.scalar.copy(out=dst, in_=psT[:, ii, :])
                        else:
                            nc.vector.tensor_copy(out=dst, in_=psT[:, ii, :])

            def emit_outproj_st(st, pools=None, split_dma=False):
                ysb = ypool.tile([128, 1024], F16, tag="ysb")
                for nn in range(2):
                    if pools is None:
                        ps = ps512.tile([128, 512], F32, tag="ps512")
                    else:
                        pool, tg = pools[nn % len(pools)]
                        ps = pool.tile([128, 512], F32, tag=tg, name="pso")
                    for hp2 in range(2):
                        nc.tensor.matmul(
                            out=ps[:],
                            lhsT=aoT_sb[:, hp2, st * 128:(st + 1) * 128],
                            rhs=wout_sb[:, hp2,
                                        nn * 512:(nn + 1) * 512],
                            start=(hp2 == 0),
                            stop=(hp2 == 1),
                        )
                    if (st * 2 + nn) % 2 == 0:
                        nc.scalar.copy(out=ysb[:, nn * 512:(nn + 1) * 512],
                                       in_=ps[:])
                    else:
                        nc.vector.tensor_copy(
                            out=ysb[:, nn * 512:(nn + 1) * 512], in_=ps[:])
                if split_dma:
                    # final tiles: halve each transfer across two engines'
                    # rings so the drain tail shrinks (~37 GB/s per ring)
                    nc.sync.dma_start(
                        out=out_d[st * 128:(st + 1) * 128, 0:512],
                        in_=ysb[:, 0:512],
                    )
                    nc.gpsimd.dma_start(
                        out=out_d[st * 128:(st + 1) * 128, 512:1024],
                        in_=ysb[:, 512:1024],
                    )
                else:
                    nc.sync.dma_start(
                        out=out_d[st * 128:(st + 1) * 128, :],
                        in_=ysb[:],
                    )

            # ---- emission schedule ----
            # prologue: q/k for seq 0:1024, v chunks for the c4=0 blocks
            rrp = [(ps512, "ps512"), (pssc, "pssc"),
                   (ps512, "ps512"), (pssc, "pssc")]
            emit_qk_chunk(0, pools=rrp)
            emit_qk_chunk(1, pools=rrp)
            nc.vector.memset(negc_sb[:], -C_SUB)
            nc.vector.memset(v_sb[:, :, :, 64:65], 1.0)
            for c in (15, 0, 1, 2, 3):
                emit_v_chunk(c)

            # per-block fillers: A runs between scores and AV^T (covers the
            # exp/mask latency AND the previous block's norm latency via its
            # transposes), B runs after AV^T (covers this block's norm).
            fillerA = {
                (0, 0): [("v", 4), ("v", 5), ("v", 6), ("v", 7)],
                (0, 1): [("qk", 2)],
                (1, 0): [("qk", 3), ("T", (0, 0))],
                (1, 1): [("T", (0, 1))],
                (2, 0): [("st", 0), ("st", 1), ("T", (1, 0))],
                (2, 1): [("st", 2), ("st", 3), ("T", (1, 1))],
                (3, 0): [("st", 4), ("st", 5), ("T", (2, 0))],
                (3, 1): [("st", 6), ("st", 7), ("T", (2, 1))],
            }
            fillerB = {
                (0, 0): [],
                (0, 1): [("v", 8), ("v", 9)],
                (1, 0): [("v", 10), ("v", 11)],
                (1, 1): [],
                (2, 0): [("v", 12), ("v", 13)],
                (2, 1): [("v", 14), ("v", 18)],
                (3, 0): [],
                (3, 1): [("st", 8), ("st", 9)],
            }
            avns = {}
            # psavT slots are drained (normed) by the time fillerA runs, so
            # outproj tiles can rotate through them alongside ps512
            trr_mid = [(ps512, "ps512"), (psavT, "psavT")]

            def run_items(items):
                for kind, it in items:
                    if kind == "qk":
                        emit_qk_chunk(it)
                    elif kind == "v":
                        emit_v_chunk(it)
                    elif kind == "vp":
                        emit_v_pair(it)
                    elif kind == "st":
                        emit_outproj_st(it, pools=trr_mid)
                    else:
                        emit_transposes(*it, avns.pop(it))

            for c4 in range(4):
                for hp in range(2):
                    ex_big = emit_scores(c4, hp)
                    run_items(fillerA[(c4, hp)])
                    avns[(c4, hp)] = emit_avT(c4, hp, ex_big)
                    run_items(fillerB[(c4, hp)])

            # tail: last two transpose groups + remaining outproj tiles.
            # psavT is drained by now — rotate its banks in so four outproj
            # PSUM groups stay open against the ACT/DVE copy latency.
            trr = [(ps512, "ps512"), (psavT, "psavT")]
            emit_transposes(3, 0, avns.pop((3, 0)))
            emit_outproj_st(10, pools=trr)
            emit_outproj_st(11, pools=trr)
            emit_transposes(3, 1, avns.pop((3, 1)), split_copy=True)
            for st in range(12, 16):
                emit_outproj_st(st, pools=trr)

    nc.compile()
    return nc


_NC = None


def _get_program():
    global _NC
    if _NC is None:
        _NC = _build_program()
    return _NC


def _make_in_maps(x, w_qkv, b_qkv, w_out):
    masks = _build_pair_masks()
    ident = np.eye(128, dtype=np.float16)

    in_maps = []
    for c in range(8):
        b, hg = divmod(c, 4)
        cq = 256 * hg
        wqk = np.concatenate(
            [w_qkv[:, cq:cq + 256], w_qkv[:, 1024 + cq:1024 + cq + 256]],
            axis=1,
        ).astype(np.float16)
        bqk = np.empty((128, 4), np.float32)
        bqk[:, 0] = b_qkv[cq:cq + 128] * SCALE
        bqk[:, 1] = b_qkv[cq + 128:cq + 256] * SCALE
        bqk[:, 2] = b_qkv[1024 + cq:1024 + cq + 128]
        bqk[:, 3] = b_qkv[1024 + cq + 128:1024 + cq + 256]
        in_maps.append({
            "xT": np.ascontiguousarray(x[b].T).astype(np.float16),
            "wqk": wqk,
            "wv": w_qkv[:, 2048 + cq:2048 + cq + 256].astype(np.float16),
            "wout": w_out[cq:cq + 256, :].astype(np.float16),
            "bqk": bqk,
            "masks": masks,
            "ident": ident,
        })
    return in_maps


def kernel(x, w_qkv, b_qkv, w_out, b_out):
    x = np.asarray(x, np.float32)
    w_qkv = np.asarray(w_qkv, np.float32)
    b_qkv = np.asarray(b_qkv, np.float32)
    w_out = np.asarray(w_out, np.float32)
    b_out = np.asarray(b_out, np.float32)

    in_maps = _make_in_maps(x, w_qkv, b_qkv, w_out)
    nc = _get_program()
    res = run_bass_kernel_spmd(nc, in_maps, list(range(8)))

    b_v = b_qkv[2048:]
    bias_all = b_out + b_v @ w_out  # folds the (untracked) v-bias
    y = np.empty((B, S, D), np.float32)
    for b in range(B):
        acc = np.zeros((S, D), np.float32)
        for hg in range(4):
            acc += res.results[4 * b + hg]["out"].astype(np.float32)
        y[b] = acc + bias_all
    return y


# revision 81
# speedup vs baseline: 1.1436x; 1.1436x over previous
"""Trainium2 Bass kernel for LocalSparseAttention.

Problem (hardcoded): B=2, S=2048, D=1024, H=16, HD=64, WINDOW=128 (band
|i-j| <= 64), fp32 I/O.

Sharding: 8 cores = 2 batches x 4 head-groups (4 heads each). Each core:
  - qk projection into transposed layout [512, 2048] (head-pair packed)
  - v projection into natural layout, 17 (possibly 64-shifted) seq chunks
    (boundary tiles reuse shifted chunks; masks dedup the overlap)
  - banded attention per 128-query tile with a 256-key window:
      scores  -> exp on ACT -> 0/1 band-mask multiply on DVE
      AV^T    : lhsT=exp tile [keys, q], rhs=v (+ ones col) -> PSUM [q, 65]
                (full 128x128 PE efficiency; denominator = ones column)
      norm    : per-partition reciprocal + stride-0-broadcast multiply
                (DVE for even head, GpSimd for odd head)
      PE transpose of normalized [q, 64] tiles back into aoT [dims, seq]
  - output projection -> fp16 partial [2048, 1024]
Host: fp16 casts/transposes in, sum of 4 partials per batch + fused bias
(b_out + b_v @ w_out) out.

All matmuls run in fp16 (1 cycle/row on PE) with fp32 PSUM accumulation;
softmax exp input stays fp32.
"""
import sys

if "/opt/trn_rl_repo" not in sys.path:
    sys.path.insert(0, "/opt/trn_rl_repo")

import numpy as np

import concourse.bass as bass
import concourse.mybir as mybir
import concourse.tile as tile
from concourse import bacc
from concourse.bass import AP
from concourse.bass_utils import run_bass_kernel_spmd

B, S, D, H, HD = 2, 2048, 1024, 16, 64
SCALE = HD**-0.5
C_SUB = 4.0  # subtracted from all scores via the exp bias; cancels in softmax

F16 = mybir.dt.float16
F32 = mybir.dt.float32

# 19 key/value chunk offsets: 15 shifted (128c+64) + aligned 0,128,1792,1920
OFFS = [128 * c + 64 for c in range(15)] + [0, 128, 1792, 1920]

N_WARMUP = 16


def _chunk_pair(i):
    # Boundary tiles reuse shifted chunks (the mask dedups the overlap),
    # so only 17 v chunks are ever materialized (15 shifted + 0 + 1920).
    if i == 0:
        return 15, 0
    if i == 15:
        return 14, 18
    return i - 1, i


def _half_mask(q_base, k_base, own_lo, own_hi):
    kp = np.arange(128)[:, None]
    q = np.arange(128)[None, :]
    k = k_base + kp
    valid = (k >= own_lo) & (k < own_hi) & (np.abs((q_base + q) - k) <= 64)
    return valid.astype(np.float16)


def _build_pair_masks():
    # variant 0: (tile 0, interior) — c4=0 pair 0
    # variant 1: (interior, interior)
    # variant 2: (interior, tile 15) — c4=3 pair 1
    # Each [kp, q] mask owns a disjoint global-key range so overlapping
    # chunk halves never double-count a key.
    mp = np.zeros((128, 3, 2, 2, 128), np.float16)
    int0 = _half_mask(128, 64, 64, 192)       # any interior tile, half 0
    int1 = _half_mask(128, 192, 192, 320)     # any interior tile, half 1
    mp[:, 1, :, 0] = int0[:, None]
    mp[:, 1, :, 1] = int1[:, None]
    mp[:, 0] = mp[:, 2] = mp[:, 1]
    mp[:, 0, 0, 0] = _half_mask(0, 0, 0, 128)      # tile 0 vs chunk 15
    mp[:, 0, 0, 1] = _half_mask(0, 64, 128, 192)   # tile 0 vs chunk 0
    mp[:, 2, 1, 0] = _half_mask(1920, 1856, 1856, 1984)  # tile 15 / chunk 14
    mp[:, 2, 1, 1] = _half_mask(1920, 1920, 1984, 2048)  # tile 15 / chunk 18
    return mp


def _bcast_free(ap, n):
    # append a stride-0 free dim of size n (broadcast along free axis)
    return AP(ap.tensor, ap.offset, list(ap.ap) + [[0, n]])


def _build_program():
    nc = bacc.Bacc("TRN2", debug=False, num_devices=8)

    xT_d = nc.dram_tensor("xT", [D, S], F16, kind="ExternalInput").ap()
    wqk_d = nc.dram_tensor("wqk", [D, 512], F16, kind="ExternalInput").ap()
    wv_d = nc.dram_tensor("wv", [D, 256], F16, kind="ExternalInput").ap()
    wout_d = nc.dram_tensor("wout", [256, D], F16, kind="ExternalInput").ap()
    bqk_d = nc.dram_tensor("bqk", [128, 4], F32, kind="ExternalInput").ap()
    masks_d = nc.dram_tensor("masks", [128, 3, 2, 2, 128], F16,
                             kind="ExternalInput").ap()
    ident_d = nc.dram_tensor("ident", [128, 128], F16,
                             kind="ExternalInput").ap()
    out_d = nc.dram_tensor("out", [S, D], F16, kind="ExternalOutput").ap()

    with tile.TileContext(nc) as tc:
        with (
            tc.tile_pool(name="const", bufs=1) as cpool,
            tc.tile_pool(name="work", bufs=2) as wpool,
            tc.tile_pool(name="expp", bufs=10) as epool,
            tc.tile_pool(name="ysb", bufs=8) as ypool,
            tc.tile_pool(name="ps512", bufs=2, space="PSUM") as ps512,
            tc.tile_pool(name="psv", bufs=2, space="PSUM") as psv,
            tc.tile_pool(name="pssc", bufs=2, space="PSUM") as pssc,
            tc.tile_pool(name="psavT", bufs=2, space="PSUM") as psavT,
        ):
            # ---- persistent SBUF tensors ----
            xT_sb = cpool.tile([128, 8, S], F16, tag="xT")
            wqk_sb = cpool.tile([128, 8, 512], F16, tag="wqk")
            wv_sb = cpool.tile([128, 8, 256], F16, tag="wv")
            wout_sb = cpool.tile([128, 2, D], F16, tag="wout")
            bqk_sb = cpool.tile([128, 4], F32, tag="bqk")
            masks_sb = cpool.tile([128, 3, 2, 2, 128], F16, tag="masks")
            ident_sb = cpool.tile([128, 128], F16, tag="ident")
            qk_sb = cpool.tile([128, 4, S], F16, tag="qk")
            v_sb = cpool.tile([128, 19, 4, 65], F16, tag="v")
            aoT_sb = cpool.tile([128, 2, S], F16, tag="aoT")
            negc_sb = cpool.tile([128, 1], F32, tag="negc")

            # ---- input DMAs: ~256KB chunks in PE-consumption order, issue
            # cost (~650ns each) split across the two HWDGE queues (sync +
            # scalar) so transfers start early and overlap across rings ----
            xT_r = xT_d.rearrange("(ko kp) s -> kp ko s", kp=128)
            wqk_r = wqk_d.rearrange("(ko kp) n -> kp ko n", kp=128)
            wv_r = wv_d.rearrange("(ko kp) n -> kp ko n", kp=128)
            wout_r = wout_d.rearrange("(t p) n -> p t n", p=128)
            # Fine-grained DMAs on the sync queue in strict consumption
            # order: small chunks land with low latency and the issue stream
            # itself paces transfers so later chunks never steal bandwidth.
            # first 16 issues alternate sync/scalar: scalar is idle until
            # ~25us (qk drains live on DVE now), and two issue queues get
            # all 8 DMA rings transferring ~2.5us sooner
            for kt in range(8):
                eng_a = nc.sync if kt % 2 == 0 else nc.scalar
                eng_b = nc.scalar if kt % 2 == 0 else nc.sync
                eng_a.dma_start(out=wqk_sb[:, kt], in_=wqk_r[:, kt])
                eng_b.dma_start(out=xT_sb[:, kt, 0:512],
                                in_=xT_r[:, kt, 0:512])
            nc.sync.dma_start(out=bqk_sb[:], in_=bqk_d)
            nc.sync.dma_start(out=masks_sb[:], in_=masks_d)
            for kt in range(8):
                nc.sync.dma_start(out=xT_sb[:, kt, 512:1024],
                                  in_=xT_r[:, kt, 512:1024])
            for kt in range(8):
                nc.sync.dma_start(out=wv_sb[:, kt], in_=wv_r[:, kt])
            nc.sync.dma_start(out=ident_sb[:], in_=ident_d)
            nc.sync.dma_start(out=wout_sb[:], in_=wout_r[:])
            for kt in range(8):
                nc.sync.dma_start(out=xT_sb[:, kt, 1024:2048],
                                  in_=xT_r[:, kt, 1024:2048])

            # ---- constants (wsrc first so the PE warmup can start ASAP;
            # negc / v-ones deferred past the prologue so DVE drains the
            # first qk PSUM groups without queueing behind them) ----
            wsrc = cpool.tile([128, 512], F16, tag="wsrc")
            nc.vector.memset(wsrc[:, 0:128], 0.0)
            nc.vector.memset(wsrc[:, 128:512], 0.0)

            # ---- PE warmup: dummy matmuls on zeroed SBUF so the HAM
            # clock-gate ramps while the first input DMAs land ----
            wdst = cpool.tile([128, 512], F16, tag="wdst")
            wps = ps512.tile([128, 512], F32, tag="ps512")
            for w in range(N_WARMUP):
                nc.tensor.matmul(
                    out=wps[:],
                    lhsT=wsrc[:, 0:128],
                    rhs=wsrc[:],
                    start=(w == 0),
                    stop=(w == N_WARMUP - 1),
                )
            nc.scalar.copy(out=wdst[:], in_=wps[:])

            # ---- emission helpers ----
            def emit_qk_chunk(ns, pools=None, on_act=False):
                # all 4 m-tiles of q/k projection for seq chunk ns.
                # on_act: drain PSUM via ACT (idle in the prologue) instead
                # of DVE (busy mid-kernel).
                for m in range(4):
                    scale = SCALE if m < 2 else 1.0
                    if pools is None:
                        ps = ps512.tile([128, 512], F32, tag="ps512")
                    else:
                        pool, tg = pools[m % len(pools)]
                        ps = pool.tile([128, 512], F32, tag=tg)
                    for kt in range(8):
                        nc.tensor.matmul(
                            out=ps[:],
                            lhsT=wqk_sb[:, kt, m * 128:(m + 1) * 128],
                            rhs=xT_sb[:, kt, ns * 512:(ns + 1) * 512],
                            start=(kt == 0),
                            stop=(kt == 7),
                        )
                    if on_act:
                        nc.scalar.activation(
                            out=qk_sb[:, m, ns * 512:(ns + 1) * 512],
                            in_=ps[:],
                            func=mybir.ActivationFunctionType.Identity,
                            bias=bqk_sb[:, m:m + 1],
                            scale=scale,
                        )
                    else:
                        nc.vector.tensor_scalar(
                            out=qk_sb[:, m, ns * 512:(ns + 1) * 512],
                            in0=ps[:],
                            scalar1=scale,
                            scalar2=bqk_sb[:, m:m + 1],
                            op0=mybir.AluOpType.mult,
                            op1=mybir.AluOpType.add,
                        )

            def emit_qk_chunk_ktmajor(ns, pools):
                # kt-major: 4 open PSUM groups, consuming each xT k-chunk
                # as its DMA lands (used for the DMA-paced prologue chunks)
                pss = []
                for m in range(4):
                    pool, tg = pools[m % len(pools)]
                    ps = pool.tile([128, 512], F32, tag=tg, name=f"qkm{m}")
                    pss.append(ps)
                for kt in range(8):
                    for m in range(4):
                        nc.tensor.matmul(
                            out=pss[m][:],
                            lhsT=wqk_sb[:, kt, m * 128:(m + 1) * 128],
                            rhs=xT_sb[:, kt, ns * 512:(ns + 1) * 512],
                            start=(kt == 0),
                            stop=(kt == 7),
                        )
                for m in range(4):
                    nc.scalar.activation(
                        out=qk_sb[:, m, ns * 512:(ns + 1) * 512],
                        in_=pss[m][:],
                        func=mybir.ActivationFunctionType.Identity,
                        bias=bqk_sb[:, m:m + 1],
                        scale=SCALE if m < 2 else 1.0,
                    )

            def emit_v_chunk(c):
                off = OFFS[c]
                ps = psv.tile([128, 256], F32, tag="psv")
                for kt in range(8):
                    nc.tensor.matmul(
                        out=ps[:],
                        lhsT=xT_sb[:, kt, off:off + 128],
                        rhs=wv_sb[:, kt, :],
                        start=(kt == 0),
                        stop=(kt == 7),
                    )
                nc.scalar.copy(
                    out=v_sb[:, c, :, 0:64],
                    in_=ps[:].rearrange("p (h d) -> p h d", h=4),
                )

            def emit_v_pair(c):
                # two adjacent chunks through one PSUM bank + one ACT copy
                ps = psv.tile([128, 2, 256], F32, tag="psv", name="psvp")
                for j in range(2):
                    off = OFFS[c + j]
                    for kt in range(8):
                        nc.tensor.matmul(
                            out=ps[:, j, :],
                            lhsT=xT_sb[:, kt, off:off + 128],
                            rhs=wv_sb[:, kt, :],
                            start=(j == 0 and kt == 0),
                            stop=(j == 1 and kt == 7),
                        )
                nc.scalar.copy(
                    out=v_sb[:, c:c + 2, :, 0:64],
                    in_=ps[:].rearrange("p c (h d) -> p c h d", h=4),
                )

            def emit_scores(c4, hp):
                # scores + exp for both heads of the pair
                ex_big0 = epool.tile([128, 4, 2, 128], F16, tag="exp")
                ex_big1 = epool.tile([128, 4, 2, 128], F16, tag="exp")
                ex_big = {0: ex_big0, 1: ex_big1}
                for pair in range(2):
                    if c4 == 0 and pair == 0:
                        pv = 0
                    elif c4 == 3 and pair == 1:
                        pv = 2
                    else:
                        pv = 1
                    sc_h0 = pssc.tile([128, 2, 2, 128], F32, tag="pssc")
                    sc_h1 = pssc.tile([128, 2, 2, 128], F32, tag="pssc")
                    scs = {0: sc_h0, 1: sc_h1}
                    for iw in range(2):
                        ii = pair * 2 + iw
                        i = c4 * 4 + ii
                        cA, cB = _chunk_pair(i)
                        for hh in range(2):
                            po = hh * 64
                            for half, cc in enumerate((cA, cB)):
                                off = OFFS[cc]
                                nc.tensor.matmul(
                                    out=scs[hh][:, iw, half, :],
                                    lhsT=qk_sb[po:po + 64, 2 + hp,
                                               off:off + 128],
                                    rhs=qk_sb[po:po + 64, hp,
                                              i * 128:(i + 1) * 128],
                                    start=(iw == 0 and half == 0),
                                    stop=(iw == 1 and half == 1),
                                )
                    for hh in range(2):
                        # exp(score - C) on ACT (one op per ii-pair),
                        # band-zeroing via 0/1 mask multiply on DVE
                        sl = slice(pair * 2, pair * 2 + 2)
                        nc.scalar.activation(
                            out=ex_big[hh][:, sl],
                            in_=scs[hh][:],
                            func=mybir.ActivationFunctionType.Exp,
                            bias=negc_sb[:],
                        )
                        nc.vector.tensor_mul(
                            out=ex_big[hh][:, sl],
                            in0=ex_big[hh][:, sl],
                            in1=masks_sb[:, pv],
                        )
                return ex_big

            def emit_avT(c4, hp, ex_big):
                # AV^T: [q, dims+1] PSUM per (ii, head); ones column of v
                # yields the softmax denominator in col 64.
                avts = {}
                for hh in range(2):
                    h = 2 * hp + hh
                    avt = psavT.tile([128, 4, 65], F32, tag="psavT",
                                     padded_shape=[128, 4, 128])
                    for ii in range(4):
                        cA, cB = _chunk_pair(c4 * 4 + ii)
                        for half, cc in enumerate((cA, cB)):
                            nc.tensor.matmul(
                                out=avt[:, ii, :],
                                lhsT=ex_big[hh][:, ii, half, :],
                                rhs=v_sb[:, cc, h, 0:65],
                                start=(ii == 0 and half == 0),
                                stop=(ii == 3 and half == 1),
                            )
                    avts[hh] = avt
                # per-query reciprocal of the denominators, then normalize
                # with a stride-0 broadcast multiply (q is on partitions)
                recip = wpool.tile([128, 2, 4], F32, tag="recip")
                # ii-major layout so each ii slice is a contiguous [128, 128]
                # holding both heads — one PE transpose covers the pair
                avn = wpool.tile([128, 4, 2, 64], F16, tag="avn")
                for hh in range(2):
                    nc.vector.reciprocal(
                        out=recip[:, hh, :],
                        in_=avts[hh][:, :, 64:65].rearrange("p a b -> p (a b)"),
                    )
                    nc.vector.tensor_mul(
                        out=avn[:, :, hh, :],
                        in0=avts[hh][:, :, 0:64],
                        in1=_bcast_free(recip[:, hh, :], 64),
                    )
                return avn

            def emit_transposes(c4, hp, avn, split_copy=False):
                # PE-transpose normalized [q, 64] tiles into aoT [dims, seq]
                psT = psv.tile([128, 4, 128], F16, tag="psv")
                for ii in range(4):
                    nc.tensor.transpose(
                        out=psT[:, ii, :],
                        in_=avn[:, ii].rearrange("p a b -> p (a b)"),
                        identity=ident_sb[:],
                    )
                sl0 = c4 * 512
                if not split_copy:
                    # DVE: its queue clears sooner than ACT's (which is
                    # backed up behind the next block's exp ops)
                    nc.vector.tensor_copy(
                        out=aoT_sb[:, hp, sl0:sl0 + 512].rearrange(
                            "p (a b) -> p a b", a=4),
                        in_=psT[:],
                    )
                else:
                    # final block: per-ii copies, alternating ACT/DVE so
                    # the last outproj tiles' inputs land two at a time
                    for ii in range(4):
                        dst = aoT_sb[:, hp, sl0 + ii * 128:sl0 + ii * 128 + 128]
                        if ii % 2 == 0:
                            nc.scalar.copy(out=dst, in_=psT[:, ii, :])
                        else:
                            nc.vector.tensor_copy(out=dst, in_=psT[:, ii, :])

            def emit_outproj_st(st, pools=None, split_dma=False):
                ysb = ypool.tile([128, 1024], F16, tag="ysb")
                for nn in range(2):
                    if pools is None:
                        ps = ps512.tile([128, 512], F32, tag="ps512")
                    else:
                        pool, tg = pools[nn % len(pools)]
                        ps = pool.tile([128, 512], F32, tag=tg, name="pso")
                    for hp2 in range(2):
                        nc.tensor.matmul(
                            out=ps[:],
                            lhsT=aoT_sb[:, hp2, st * 128:(st + 1) * 128],
                            rhs=wout_sb[:, hp2,
                                        nn * 512:(nn + 1) * 512],
                            start=(hp2 == 0),
                            stop=(hp2 == 1),
                        )
                    if (st * 2 + nn) % 2 == 0:
                        nc.scalar.copy(out=ysb[:, nn * 512:(nn + 1) * 512],
                                       in_=ps[:])
                    else:
                        nc.vector.tensor_copy(
                            out=ysb[:, nn * 512:(nn + 1) * 512], in_=ps[:])
                if split_dma:
                    # final tiles: halve each transfer across two engines'
                    # rings so the drain tail shrinks (~37 GB/s per ring)
                    nc.sync.dma_start(
                        out=out_d[st * 128:(st + 1) * 128, 0:512],
                        in_=ysb[:, 0:512],
                    )
                    nc.gpsimd.dma_start(
                        out=out_d[st * 128:(st + 1) * 128, 512:1024],
                        in_=ysb[:, 512:1024],
                    )
                else:
                    nc.sync.dma_start(
                        out=out_d[st * 128:(st + 1) * 128, :],
                        in_=ysb[:],
                    )

            # ---- emission schedule ----
            # prologue: q/k for seq 0:1024, v chunks for the c4=0 blocks
            rrp = [(ps512, "ps512"), (pssc, "pssc"),
                   (ps512, "ps512"), (pssc, "pssc")]
            emit_qk_chunk(0, pools=rrp)
            emit_qk_chunk(1, pools=rrp)
            nc.vector.memset(negc_sb[:], -C_SUB)
            nc.vector.memset(v_sb[:, :, :, 64:65], 1.0)
            for c in (15, 0, 1, 2, 3):
                emit_v_chunk(c)

            # per-block fillers: A runs between scores and AV^T (covers the
            # exp/mask latency AND the previous block's norm latency via its
            # transposes), B runs after AV^T (covers this block's norm).
            fillerA = {
                (0, 0): [("v", 4), ("v", 5), ("v", 6), ("v", 7)],
                (0, 1): [("qk", 2)],
                (1, 0): [("qk", 3), ("T", (0, 0))],
                (1, 1): [("T", (0, 1))],
                (2, 0): [("st", 0), ("st", 1), ("T", (1, 0))],
                (2, 1): [("st", 2), ("st", 3), ("T", (1, 1))],
                (3, 0): [("st", 4), ("st", 5), ("T", (2, 0))],
                (3, 1): [("st", 6), ("st", 7), ("T", (2, 1))],
            }
            fillerB = {
                (0, 0): [],
                (0, 1): [("v", 8), ("v", 9)],
                (1, 0): [("v", 10), ("v", 11)],
                (1, 1): [],
                (2, 0): [("v", 12), ("v", 13)],
                (2, 1): [("v", 14), ("v", 18)],
                (3, 0): [],
                (3, 1): [("st", 8), ("st", 9)],
            }
            avns = {}
            # psavT slots are drained (normed) by the time fillerA runs, so
            # outproj tiles can rotate through them alongside ps512
            trr_mid = [(ps512, "ps512"), (psavT, "psavT")]

            def run_items(items):
                for kind, it in items:
                    if kind == "qk":
                        emit_qk_chunk(it)
                    elif kind == "v":
                        emit_v_chunk(it)
                    elif kind == "vp":
                        emit_v_pair(it)
                    elif kind == "st":
                        emit_outproj_st(it, pools=trr_mid)
                    else:
                        emit_transposes(*it, avns.pop(it))

            for c4 in range(4):
                for hp in range(2):
                    ex_big = emit_scores(c4, hp)
                    run_items(fillerA[(c4, hp)])
                    avns[(c4, hp)] = emit_avT(c4, hp, ex_big)
                    run_items(fillerB[(c4, hp)])

            # tail: last two transpose groups + remaining outproj tiles.
            # psavT is drained by now — rotate its banks in so four outproj
            # PSUM groups stay open against the ACT/DVE copy latency.
            trr = [(ps512, "ps512"), (psavT, "psavT")]
            emit_transposes(3, 0, avns.pop((3, 0)))
            emit_outproj_st(10, pools=trr)
            emit_outproj_st(11, pools=trr)
            emit_transposes(3, 1, avns.pop((3, 1)), split_copy=True)
            for st in range(12, 16):
                emit_outproj_st(st, pools=trr)

    nc.compile()
    return nc


_NC = None


def _get_program():
    global _NC
    if _NC is None:
        _NC = _build_program()
    return _NC


def _make_in_maps(x, w_qkv, b_qkv, w_out):
    masks = _build_pair_masks()
    ident = np.eye(128, dtype=np.float16)

    in_maps = []
    for c in range(8):
        b, hg = divmod(c, 4)
        cq = 256 * hg
        wqk = np.concatenate(
            [w_qkv[:, cq:cq + 256], w_qkv[:, 1024 + cq:1024 + cq + 256]],
            axis=1,
        ).astype(np.float16)
        bqk = np.empty((128, 4), np.float32)
        bqk[:, 0] = b_qkv[cq:cq + 128] * SCALE
        bqk[:, 1] = b_qkv[cq + 128:cq + 256] * SCALE
        bqk[:, 2] = b_qkv[1024 + cq:1024 + cq + 128]
        bqk[:, 3] = b_qkv[1024 + cq + 128:1024 + cq + 256]
        in_maps.append({
            "xT": np.ascontiguousarray(x[b].T).astype(np.float16),
            "wqk": wqk,
            "wv": w_qkv[:, 2048 + cq:2048 + cq + 256].astype(np.float16),
            "wout": w_out[cq:cq + 256, :].astype(np.float16),
            "bqk": bqk,
            "masks": masks,
            "ident": ident,
        })
    return in_maps


def kernel(x, w_qkv, b_qkv, w_out, b_out):
    x = np.asarray(x, np.float32)
    w_qkv = np.asarray(w_qkv, np.float32)
    b_qkv = np.asarray(b_qkv, np.float32)
    w_out = np.asarray(w_out, np.float32)
    b_out = np.asarray(b_out, np.float32)

    in_maps = _make_in_maps(x, w_qkv, b_qkv, w_out)
    nc = _get_program()
    res = run_bass_kernel_spmd(nc, in_maps, list(range(8)))

    b_v = b_qkv[2048:]
    bias_all = b_out + b_v @ w_out  # folds the (untracked) v-bias
    y = np.empty((B, S, D), np.float32)
    for b in range(B):
        acc = np.zeros((S, D), np.float32)
        for hg in range(4):
            acc += res.results[4 * b + hg]["out"].astype(np.float32)
        y[b] = acc + bias_all
    return y


# revision 82
# speedup vs baseline: 1.1634x; 1.0173x over previous
"""Trainium2 Bass kernel for LocalSparseAttention.

Problem (hardcoded): B=2, S=2048, D=1024, H=16, HD=64, WINDOW=128 (band
|i-j| <= 64), fp32 I/O.

Sharding: 8 cores = 2 batches x 4 head-groups (4 heads each). Each core:
  - qk projection into transposed layout [512, 2048] (head-pair packed)
  - v projection into natural layout, 17 (possibly 64-shifted) seq chunks
    (boundary tiles reuse shifted chunks; masks dedup the overlap)
  - banded attention per 128-query tile with a 256-key window:
      scores  -> exp on ACT -> 0/1 band-mask multiply on DVE
      AV^T    : lhsT=exp tile [keys, q], rhs=v (+ ones col) -> PSUM [q, 65]
                (full 128x128 PE efficiency; denominator = ones column)
      norm    : per-partition reciprocal + stride-0-broadcast multiply
                (DVE for even head, GpSimd for odd head)
      PE transpose of normalized [q, 64] tiles back into aoT [dims, seq]
  - output projection -> fp16 partial [2048, 1024]
Host: fp16 casts/transposes in, sum of 4 partials per batch + fused bias
(b_out + b_v @ w_out) out.

All matmuls run in fp16 (1 cycle/row on PE) with fp32 PSUM accumulation;
softmax exp input stays fp32.
"""
import sys

if "/opt/trn_rl_repo" not in sys.path:
    sys.path.insert(0, "/opt/trn_rl_repo")

import numpy as np

import concourse.bass as bass
import concourse.mybir as mybir
import concourse.tile as tile
from concourse import bacc
from concourse.bass import AP
from concourse.bass_utils import run_bass_kernel_spmd

B, S, D, H, HD = 2, 2048, 1024, 16, 64
SCALE = HD**-0.5
C_SUB = 4.0  # subtracted from all scores via the exp bias; cancels in softmax

F16 = mybir.dt.float16
F32 = mybir.dt.float32

# 19 key/value chunk offsets: 15 shifted (128c+64) + aligned 0,128,1792,1920
OFFS = [128 * c + 64 for c in range(15)] + [0, 128, 1792, 1920]

N_WARMUP = 18


def _chunk_pair(i):
    # Boundary tiles reuse shifted chunks (the mask dedups the overlap),
    # so only 17 v chunks are ever materialized (15 shifted + 0 + 1920).
    if i == 0:
        return 15, 0
    if i == 15:
        return 14, 18
    return i - 1, i


def _half_mask(q_base, k_base, own_lo, own_hi):
    kp = np.arange(128)[:, None]
    q = np.arange(128)[None, :]
    k = k_base + kp
    valid = (k >= own_lo) & (k < own_hi) & (np.abs((q_base + q) - k) <= 64)
    return valid.astype(np.float16)


def _build_pair_masks():
    # variant 0: (tile 0, interior) — c4=0 pair 0
    # variant 1: (interior, interior)
    # variant 2: (interior, tile 15) — c4=3 pair 1
    # Each [kp, q] mask owns a disjoint global-key range so overlapping
    # chunk halves never double-count a key.
    mp = np.zeros((128, 3, 2, 2, 128), np.float16)
    int0 = _half_mask(128, 64, 64, 192)       # any interior tile, half 0
    int1 = _half_mask(128, 192, 192, 320)     # any interior tile, half 1
    mp[:, 1, :, 0] = int0[:, None]
    mp[:, 1, :, 1] = int1[:, None]
    mp[:, 0] = mp[:, 2] = mp[:, 1]
    mp[:, 0, 0, 0] = _half_mask(0, 0, 0, 128)      # tile 0 vs chunk 15
    mp[:, 0, 0, 1] = _half_mask(0, 64, 128, 192)   # tile 0 vs chunk 0
    mp[:, 2, 1, 0] = _half_mask(1920, 1856, 1856, 1984)  # tile 15 / chunk 14
    mp[:, 2, 1, 1] = _half_mask(1920, 1920, 1984, 2048)  # tile 15 / chunk 18
    return mp


def _bcast_free(ap, n):
    # append a stride-0 free dim of size n (broadcast along free axis)
    return AP(ap.tensor, ap.offset, list(ap.ap) + [[0, n]])


def _build_program():
    nc = bacc.Bacc("TRN2", debug=False, num_devices=8)

    xT_d = nc.dram_tensor("xT", [D, S], F16, kind="ExternalInput").ap()
    wqk_d = nc.dram_tensor("wqk", [D, 512], F16, kind="ExternalInput").ap()
    wv_d = nc.dram_tensor("wv", [D, 256], F16, kind="ExternalInput").ap()
    wout_d = nc.dram_tensor("wout", [256, D], F16, kind="ExternalInput").ap()
    bqk_d = nc.dram_tensor("bqk", [128, 4], F32, kind="ExternalInput").ap()
    masks_d = nc.dram_tensor("masks", [128, 3, 2, 2, 128], F16,
                             kind="ExternalInput").ap()
    ident_d = nc.dram_tensor("ident", [128, 128], F16,
                             kind="ExternalInput").ap()
    out_d = nc.dram_tensor("out", [S, D], F16, kind="ExternalOutput").ap()

    with tile.TileContext(nc) as tc:
        with (
            tc.tile_pool(name="const", bufs=1) as cpool,
            tc.tile_pool(name="work", bufs=2) as wpool,
            tc.tile_pool(name="expp", bufs=10) as epool,
            tc.tile_pool(name="ysb", bufs=8) as ypool,
            tc.tile_pool(name="ps512", bufs=2, space="PSUM") as ps512,
            tc.tile_pool(name="psv", bufs=2, space="PSUM") as psv,
            tc.tile_pool(name="pssc", bufs=2, space="PSUM") as pssc,
            tc.tile_pool(name="psavT", bufs=2, space="PSUM") as psavT,
        ):
            # ---- persistent SBUF tensors ----
            xT_sb = cpool.tile([128, 8, S], F16, tag="xT")
            wqk_sb = cpool.tile([128, 8, 512], F16, tag="wqk")
            wv_sb = cpool.tile([128, 8, 256], F16, tag="wv")
            wout_sb = cpool.tile([128, 2, D], F16, tag="wout")
            bqk_sb = cpool.tile([128, 4], F32, tag="bqk")
            masks_sb = cpool.tile([128, 3, 2, 2, 128], F16, tag="masks")
            ident_sb = cpool.tile([128, 128], F16, tag="ident")
            qk_sb = cpool.tile([128, 4, S], F16, tag="qk")
            v_sb = cpool.tile([128, 19, 4, 65], F16, tag="v")
            aoT_sb = cpool.tile([128, 2, S], F16, tag="aoT")
            negc_sb = cpool.tile([128, 1], F32, tag="negc")

            # ---- input DMAs: ~256KB chunks in PE-consumption order, issue
            # cost (~650ns each) split across the two HWDGE queues (sync +
            # scalar) so transfers start early and overlap across rings ----
            xT_r = xT_d.rearrange("(ko kp) s -> kp ko s", kp=128)
            wqk_r = wqk_d.rearrange("(ko kp) n -> kp ko n", kp=128)
            wv_r = wv_d.rearrange("(ko kp) n -> kp ko n", kp=128)
            wout_r = wout_d.rearrange("(t p) n -> p t n", p=128)
            # Fine-grained DMAs on the sync queue in strict consumption
            # order: small chunks land with low latency and the issue stream
            # itself paces transfers so later chunks never steal bandwidth.
            # first 16 issues alternate sync/scalar: scalar is idle until
            # ~25us (qk drains live on DVE now), and two issue queues get
            # all 8 DMA rings transferring ~2.5us sooner
            for kt in range(8):
                eng_a = nc.sync if kt % 2 == 0 else nc.scalar
                eng_b = nc.scalar if kt % 2 == 0 else nc.sync
                eng_a.dma_start(out=wqk_sb[:, kt], in_=wqk_r[:, kt])
                eng_b.dma_start(out=xT_sb[:, kt, 0:512],
                                in_=xT_r[:, kt, 0:512])
            nc.sync.dma_start(out=bqk_sb[:], in_=bqk_d)
            nc.sync.dma_start(out=masks_sb[:], in_=masks_d)
            for kt in range(8):
                nc.sync.dma_start(out=xT_sb[:, kt, 512:1024],
                                  in_=xT_r[:, kt, 512:1024])
            for kt in range(8):
                nc.sync.dma_start(out=wv_sb[:, kt], in_=wv_r[:, kt])
            nc.sync.dma_start(out=ident_sb[:], in_=ident_d)
            nc.sync.dma_start(out=wout_sb[:], in_=wout_r[:])
            for kt in range(8):
                nc.sync.dma_start(out=xT_sb[:, kt, 1024:2048],
                                  in_=xT_r[:, kt, 1024:2048])

            # ---- constants (wsrc first so the PE warmup can start ASAP;
            # negc / v-ones deferred past the prologue so DVE drains the
            # first qk PSUM groups without queueing behind them) ----
            wsrc = cpool.tile([128, 512], F16, tag="wsrc")
            nc.vector.memset(wsrc[:, 0:128], 0.0)
            nc.vector.memset(wsrc[:, 128:512], 0.0)

            # ---- PE warmup: dummy matmuls on zeroed SBUF so the HAM
            # clock-gate ramps while the first input DMAs land ----
            wdst = cpool.tile([128, 512], F16, tag="wdst")
            wps = ps512.tile([128, 512], F32, tag="ps512")
            for w in range(N_WARMUP):
                nc.tensor.matmul(
                    out=wps[:],
                    lhsT=wsrc[:, 0:128],
                    rhs=wsrc[:],
                    start=(w == 0),
                    stop=(w == N_WARMUP - 1),
                )
            nc.scalar.copy(out=wdst[:], in_=wps[:])

            # ---- emission helpers ----
            def emit_qk_chunk(ns, pools=None, on_act=False):
                # all 4 m-tiles of q/k projection for seq chunk ns.
                # on_act: drain PSUM via ACT (idle in the prologue) instead
                # of DVE (busy mid-kernel).
                for m in range(4):
                    scale = SCALE if m < 2 else 1.0
                    if pools is None:
                        ps = ps512.tile([128, 512], F32, tag="ps512")
                    else:
                        pool, tg = pools[m % len(pools)]
                        ps = pool.tile([128, 512], F32, tag=tg)
                    for kt in range(8):
                        nc.tensor.matmul(
                            out=ps[:],
                            lhsT=wqk_sb[:, kt, m * 128:(m + 1) * 128],
                            rhs=xT_sb[:, kt, ns * 512:(ns + 1) * 512],
                            start=(kt == 0),
                            stop=(kt == 7),
                        )
                    if on_act:
                        nc.scalar.activation(
                            out=qk_sb[:, m, ns * 512:(ns + 1) * 512],
                            in_=ps[:],
                            func=mybir.ActivationFunctionType.Identity,
                            bias=bqk_sb[:, m:m + 1],
                            scale=scale,
                        )
                    else:
                        nc.vector.tensor_scalar(
                            out=qk_sb[:, m, ns * 512:(ns + 1) * 512],
                            in0=ps[:],
                            scalar1=scale,
                            scalar2=bqk_sb[:, m:m + 1],
                            op0=mybir.AluOpType.mult,
                            op1=mybir.AluOpType.add,
                        )

            def emit_qk_chunk_ktmajor(ns, pools):
                # kt-major: 4 open PSUM groups, consuming each xT k-chunk
                # as its DMA lands (used for the DMA-paced prologue chunks)
                pss = []
                for m in range(4):
                    pool, tg = pools[m % len(pools)]
                    ps = pool.tile([128, 512], F32, tag=tg, name=f"qkm{m}")
                    pss.append(ps)
                for kt in range(8):
                    for m in range(4):
                        nc.tensor.matmul(
                            out=pss[m][:],
                            lhsT=wqk_sb[:, kt, m * 128:(m + 1) * 128],
                            rhs=xT_sb[:, kt, ns * 512:(ns + 1) * 512],
                            start=(kt == 0),
                            stop=(kt == 7),
                        )
                for m in range(4):
                    nc.scalar.activation(
                        out=qk_sb[:, m, ns * 512:(ns + 1) * 512],
                        in_=pss[m][:],
                        func=mybir.ActivationFunctionType.Identity,
                        bias=bqk_sb[:, m:m + 1],
                        scale=SCALE if m < 2 else 1.0,
                    )

            def emit_v_chunk(c):
                off = OFFS[c]
                ps = psv.tile([128, 256], F32, tag="psv")
                for kt in range(8):
                    nc.tensor.matmul(
                        out=ps[:],
                        lhsT=xT_sb[:, kt, off:off + 128],
                        rhs=wv_sb[:, kt, :],
                        start=(kt == 0),
                        stop=(kt == 7),
                    )
                nc.scalar.copy(
                    out=v_sb[:, c, :, 0:64],
                    in_=ps[:].rearrange("p (h d) -> p h d", h=4),
                )

            def emit_v_pair(c):
                # two adjacent chunks through one PSUM bank + one ACT copy
                ps = psv.tile([128, 2, 256], F32, tag="psv", name="psvp")
                for j in range(2):
                    off = OFFS[c + j]
                    for kt in range(8):
                        nc.tensor.matmul(
                            out=ps[:, j, :],
                            lhsT=xT_sb[:, kt, off:off + 128],
                            rhs=wv_sb[:, kt, :],
                            start=(j == 0 and kt == 0),
                            stop=(j == 1 and kt == 7),
                        )
                nc.scalar.copy(
                    out=v_sb[:, c:c + 2, :, 0:64],
                    in_=ps[:].rearrange("p c (h d) -> p c h d", h=4),
                )

            def emit_scores(c4, hp):
                # scores + exp for both heads of the pair
                ex_big0 = epool.tile([128, 4, 2, 128], F16, tag="exp")
                ex_big1 = epool.tile([128, 4, 2, 128], F16, tag="exp")
                ex_big = {0: ex_big0, 1: ex_big1}
                for pair in range(2):
                    if c4 == 0 and pair == 0:
                        pv = 0
                    elif c4 == 3 and pair == 1:
                        pv = 2
                    else:
                        pv = 1
                    sc_h0 = pssc.tile([128, 2, 2, 128], F32, tag="pssc")
                    sc_h1 = pssc.tile([128, 2, 2, 128], F32, tag="pssc")
                    scs = {0: sc_h0, 1: sc_h1}
                    for iw in range(2):
                        ii = pair * 2 + iw
                        i = c4 * 4 + ii
                        cA, cB = _chunk_pair(i)
                        for hh in range(2):
                            po = hh * 64
                            for half, cc in enumerate((cA, cB)):
                                off = OFFS[cc]
                                nc.tensor.matmul(
                                    out=scs[hh][:, iw, half, :],
                                    lhsT=qk_sb[po:po + 64, 2 + hp,
                                               off:off + 128],
                                    rhs=qk_sb[po:po + 64, hp,
                                              i * 128:(i + 1) * 128],
                                    start=(iw == 0 and half == 0),
                                    stop=(iw == 1 and half == 1),
                                )
                    for hh in range(2):
                        # exp(score - C) on ACT (one op per ii-pair),
                        # band-zeroing via 0/1 mask multiply on DVE
                        sl = slice(pair * 2, pair * 2 + 2)
                        nc.scalar.activation(
                            out=ex_big[hh][:, sl],
                            in_=scs[hh][:],
                            func=mybir.ActivationFunctionType.Exp,
                            bias=negc_sb[:],
                        )
                        nc.vector.tensor_mul(
                            out=ex_big[hh][:, sl],
                            in0=ex_big[hh][:, sl],
                            in1=masks_sb[:, pv],
                        )
                return ex_big

            def emit_avT(c4, hp, ex_big):
                # AV^T: [q, dims+1] PSUM per (ii, head); ones column of v
                # yields the softmax denominator in col 64.
                avts = {}
                for hh in range(2):
                    h = 2 * hp + hh
                    avt = psavT.tile([128, 4, 65], F32, tag="psavT",
                                     padded_shape=[128, 4, 128])
                    for ii in range(4):
                        cA, cB = _chunk_pair(c4 * 4 + ii)
                        for half, cc in enumerate((cA, cB)):
                            nc.tensor.matmul(
                                out=avt[:, ii, :],
                                lhsT=ex_big[hh][:, ii, half, :],
                                rhs=v_sb[:, cc, h, 0:65],
                                start=(ii == 0 and half == 0),
                                stop=(ii == 3 and half == 1),
                            )
                    avts[hh] = avt
                # per-query reciprocal of the denominators, then normalize
                # with a stride-0 broadcast multiply (q is on partitions)
                recip = wpool.tile([128, 2, 4], F32, tag="recip")
                # ii-major layout so each ii slice is a contiguous [128, 128]
                # holding both heads — one PE transpose covers the pair
                avn = wpool.tile([128, 4, 2, 64], F16, tag="avn")
                for hh in range(2):
                    nc.vector.reciprocal(
                        out=recip[:, hh, :],
                        in_=avts[hh][:, :, 64:65].rearrange("p a b -> p (a b)"),
                    )
                    nc.vector.tensor_mul(
                        out=avn[:, :, hh, :],
                        in0=avts[hh][:, :, 0:64],
                        in1=_bcast_free(recip[:, hh, :], 64),
                    )
                return avn

            def emit_transposes(c4, hp, avn, split_copy=False):
                # PE-transpose normalized [q, 64] tiles into aoT [dims, seq]
                psT = psv.tile([128, 4, 128], F16, tag="psv")
                for ii in range(4):
                    nc.tensor.transpose(
                        out=psT[:, ii, :],
                        in_=avn[:, ii].rearrange("p a b -> p (a b)"),
                        identity=ident_sb[:],
                    )
                sl0 = c4 * 512
                if not split_copy:
                    # DVE: its queue clears sooner than ACT's (which is
                    # backed up behind the next block's exp ops)
                    nc.vector.tensor_copy(
                        out=aoT_sb[:, hp, sl0:sl0 + 512].rearrange(
                            "p (a b) -> p a b", a=4),
                        in_=psT[:],
                    )
                else:
                    # final block: per-ii copies, alternating ACT/DVE so
                    # the last outproj tiles' inputs land two at a time
                    for ii in range(4):
                        dst = aoT_sb[:, hp, sl0 + ii * 128:sl0 + ii * 128 + 128]
                        if ii % 2 == 0:
                            nc.scalar.copy(out=dst, in_=psT[:, ii, :])
                        else:
                            nc.vector.tensor_copy(out=dst, in_=psT[:, ii, :])

            def emit_outproj_st(st, pools=None, split_dma=False):
                ysb = ypool.tile([128, 1024], F16, tag="ysb")
                for nn in range(2):
                    if pools is None:
                        ps = ps512.tile([128, 512], F32, tag="ps512")
                    else:
                        pool, tg = pools[nn % len(pools)]
                        ps = pool.tile([128, 512], F32, tag=tg, name="pso")
                    for hp2 in range(2):
                        nc.tensor.matmul(
                            out=ps[:],
                            lhsT=aoT_sb[:, hp2, st * 128:(st + 1) * 128],
                            rhs=wout_sb[:, hp2,
                                        nn * 512:(nn + 1) * 512],
                            start=(hp2 == 0),
                            stop=(hp2 == 1),
                        )
                    if (st * 2 + nn) % 2 == 0:
                        nc.scalar.copy(out=ysb[:, nn * 512:(nn + 1) * 512],
                                       in_=ps[:])
                    else:
                        nc.vector.tensor_copy(
                            out=ysb[:, nn * 512:(nn + 1) * 512], in_=ps[:])
                if split_dma:
                    # final tiles: halve each transfer across two engines'
                    # rings so the drain tail shrinks (~37 GB/s per ring)
                    nc.sync.dma_start(
                        out=out_d[st * 128:(st + 1) * 128, 0:512],
                        in_=ysb[:, 0:512],
                    )
                    nc.gpsimd.dma_start(
                        out=out_d[st * 128:(st + 1) * 128, 512:1024],
                        in_=ysb[:, 512:1024],
                    )
                else:
                    nc.sync.dma_start(
                        out=out_d[st * 128:(st + 1) * 128, :],
                        in_=ysb[:],
                    )

            # ---- emission schedule ----
            # prologue: q/k for seq 0:1024, v chunks for the c4=0 blocks
            rrp = [(ps512, "ps512"), (pssc, "pssc"),
                   (ps512, "ps512"), (pssc, "pssc")]
            emit_qk_chunk(0, pools=rrp)
            emit_qk_chunk(1, pools=rrp)
            nc.vector.memset(negc_sb[:], -C_SUB)
            nc.vector.memset(v_sb[:, :, :, 64:65], 1.0)
            for c in (15, 0, 1, 2, 3):
                emit_v_chunk(c)

            # per-block fillers: A runs between scores and AV^T (covers the
            # exp/mask latency AND the previous block's norm latency via its
            # transposes), B runs after AV^T (covers this block's norm).
            fillerA = {
                (0, 0): [("v", 4), ("v", 5), ("v", 6), ("v", 7)],
                (0, 1): [("qk", 2)],
                (1, 0): [("qk", 3), ("T", (0, 0))],
                (1, 1): [("T", (0, 1))],
                (2, 0): [("st", 0), ("st", 1), ("T", (1, 0))],
                (2, 1): [("st", 2), ("st", 3), ("T", (1, 1))],
                (3, 0): [("st", 4), ("st", 5), ("T", (2, 0))],
                (3, 1): [("st", 6), ("st", 7), ("T", (2, 1))],
            }
            fillerB = {
                (0, 0): [],
                (0, 1): [("v", 8), ("v", 9)],
                (1, 0): [("v", 10), ("v", 11)],
                (1, 1): [],
                (2, 0): [("v", 12), ("v", 13)],
                (2, 1): [("v", 14), ("v", 18)],
                (3, 0): [],
                (3, 1): [("st", 8), ("st", 9)],
            }
            avns = {}
            # psavT slots are drained (normed) by the time fillerA runs, so
            # outproj tiles can rotate through them alongside ps512
            trr_mid = [(ps512, "ps512"), (psavT, "psavT")]

            def run_items(items):
                for kind, it in items:
                    if kind == "qk":
                        emit_qk_chunk(it)
                    elif kind == "v":
                        emit_v_chunk(it)
                    elif kind == "vp":
                        emit_v_pair(it)
                    elif kind == "st":
                        emit_outproj_st(it, pools=trr_mid)
                    else:
                        emit_transposes(*it, avns.pop(it))

            for c4 in range(4):
                for hp in range(2):
                    ex_big = emit_scores(c4, hp)
                    run_items(fillerA[(c4, hp)])
                    avns[(c4, hp)] = emit_avT(c4, hp, ex_big)
                    run_items(fillerB[(c4, hp)])

            # tail: last two transpose groups + remaining outproj tiles.
            # psavT is drained by now — rotate its banks in so four outproj
            # PSUM groups stay open against the ACT/DVE copy latency.
            trr = [(ps512, "ps512"), (psavT, "psavT")]
            emit_transposes(3, 0, avns.pop((3, 0)))
            emit_outproj_st(10, pools=trr)
            emit_outproj_st(11, pools=trr)
            emit_transposes(3, 1, avns.pop((3, 1)), split_copy=True)
            for st in range(12, 16):
                emit_outproj_st(st, pools=trr)

    nc.compile()
    return nc


_NC = None


def _get_program():
    global _NC
    if _NC is None:
        _NC = _build_program()
    return _NC


def _make_in_maps(x, w_qkv, b_qkv, w_out):
    masks = _build_pair_masks()
    ident = np.eye(128, dtype=np.float16)

    in_maps = []
    for c in range(8):
        b, hg = divmod(c, 4)
        cq = 256 * hg
        wqk = np.concatenate(
            [w_qkv[:, cq:cq + 256], w_qkv[:, 1024 + cq:1024 + cq + 256]],
            axis=1,
        ).astype(np.float16)
        bqk = np.empty((128, 4), np.float32)
        bqk[:, 0] = b_qkv[cq:cq + 128] * SCALE
        bqk[:, 1] = b_qkv[cq + 128:cq + 256] * SCALE
        bqk[:, 2] = b_qkv[1024 + cq:1024 + cq + 128]
        bqk[:, 3] = b_qkv[1024 + cq + 128:1024 + cq + 256]
        in_maps.append({
            "xT": np.ascontiguousarray(x[b].T).astype(np.float16),
            "wqk": wqk,
            "wv": w_qkv[:, 2048 + cq:2048 + cq + 256].astype(np.float16),
            "wout": w_out[cq:cq + 256, :].astype(np.float16),
            "bqk": bqk,
            "masks": masks,
            "ident": ident,
        })
    return in_maps


def kernel(x, w_qkv, b_qkv, w_out, b_out):
    x = np.asarray(x, np.float32)
    w_qkv = np.asarray(w_qkv, np.float32)
    b_qkv = np.asarray(b_qkv, np.float32)
    w_out = np.asarray(w_out, np.float32)
    b_out = np.asarray(b_out, np.float32)

    in_maps = _make_in_maps(x, w_qkv, b_qkv, w_out)
    nc = _get_program()
    res = run_bass_kernel_spmd(nc, in_maps, list(range(8)))

    b_v = b_qkv[2048:]
    bias_all = b_out + b_v @ w_out  # folds the (untracked) v-bias
    y = np.empty((B, S, D), np.float32)
    for b in range(B):
        acc = np.zeros((S, D), np.float32)
        for hg in range(4):
            acc += res.results[4 * b + hg]["out"].astype(np.float32)
        y[b] = acc + bias_all
    return y


# revision 83
# speedup vs baseline: 1.1694x; 1.0052x over previous
"""Trainium2 Bass kernel for LocalSparseAttention.

Problem (hardcoded): B=2, S=2048, D=1024, H=16, HD=64, WINDOW=128 (band
|i-j| <= 64), fp32 I/O.

Sharding: 8 cores = 2 batches x 4 head-groups (4 heads each). Each core:
  - qk projection into transposed layout [512, 2048] (head-pair packed)
  - v projection into natural layout, 17 (possibly 64-shifted) seq chunks
    (boundary tiles reuse shifted chunks; masks dedup the overlap)
  - banded attention per 128-query tile with a 256-key window:
      scores  -> exp on ACT -> 0/1 band-mask multiply on DVE
      AV^T    : lhsT=exp tile [keys, q], rhs=v (+ ones col) -> PSUM [q, 65]
                (full 128x128 PE efficiency; denominator = ones column)
      norm    : per-partition reciprocal + stride-0-broadcast multiply
                (DVE for even head, GpSimd for odd head)
      PE transpose of normalized [q, 64] tiles back into aoT [dims, seq]
  - output projection -> fp16 partial [2048, 1024]
Host: fp16 casts/transposes in, sum of 4 partials per batch + fused bias
(b_out + b_v @ w_out) out.

All matmuls run in fp16 (1 cycle/row on PE) with fp32 PSUM accumulation;
softmax exp input stays fp32.
"""
import sys

if "/opt/trn_rl_repo" not in sys.path:
    sys.path.insert(0, "/opt/trn_rl_repo")

import numpy as np

import concourse.bass as bass
import concourse.mybir as mybir
import concourse.tile as tile
from concourse import bacc
from concourse.bass import AP
from concourse.bass_utils import run_bass_kernel_spmd

B, S, D, H, HD = 2, 2048, 1024, 16, 64
SCALE = HD**-0.5
C_SUB = 4.0  # subtracted from all scores via the exp bias; cancels in softmax

F16 = mybir.dt.float16
F32 = mybir.dt.float32

# 19 key/value chunk offsets: 15 shifted (128c+64) + aligned 0,128,1792,1920
OFFS = [128 * c + 64 for c in range(15)] + [0, 128, 1792, 1920]

N_WARMUP = 20


def _chunk_pair(i):
    # Boundary tiles reuse shifted chunks (the mask dedups the overlap),
    # so only 17 v chunks are ever materialized (15 shifted + 0 + 1920).
    if i == 0:
        return 15, 0
    if i == 15:
        return 14, 18
    return i - 1, i


def _half_mask(q_base, k_base, own_lo, own_hi):
    kp = np.arange(128)[:, None]
    q = np.arange(128)[None, :]
    k = k_base + kp
    valid = (k >= own_lo) & (k < own_hi) & (np.abs((q_base + q) - k) <= 64)
    return valid.astype(np.float16)


def _build_pair_masks():
    # variant 0: (tile 0, interior) — c4=0 pair 0
    # variant 1: (interior, interior)
    # variant 2: (interior, tile 15) — c4=3 pair 1
    # Each [kp, q] mask owns a disjoint global-key range so overlapping
    # chunk halves never double-count a key.
    mp = np.zeros((128, 3, 2, 2, 128), np.float16)
    int0 = _half_mask(128, 64, 64, 192)       # any interior tile, half 0
    int1 = _half_mask(128, 192, 192, 320)     # any interior tile, half 1
    mp[:, 1, :, 0] = int0[:, None]
    mp[:, 1, :, 1] = int1[:, None]
    mp[:, 0] = mp[:, 2] = mp[:, 1]
    mp[:, 0, 0, 0] = _half_mask(0, 0, 0, 128)      # tile 0 vs chunk 15
    mp[:, 0, 0, 1] = _half_mask(0, 64, 128, 192)   # tile 0 vs chunk 0
    mp[:, 2, 1, 0] = _half_mask(1920, 1856, 1856, 1984)  # tile 15 / chunk 14
    mp[:, 2, 1, 1] = _half_mask(1920, 1920, 1984, 2048)  # tile 15 / chunk 18
    return mp


def _bcast_free(ap, n):
    # append a stride-0 free dim of size n (broadcast along free axis)
    return AP(ap.tensor, ap.offset, list(ap.ap) + [[0, n]])


def _build_program():
    nc = bacc.Bacc("TRN2", debug=False, num_devices=8)

    xT_d = nc.dram_tensor("xT", [D, S], F16, kind="ExternalInput").ap()
    wqk_d = nc.dram_tensor("wqk", [D, 512], F16, kind="ExternalInput").ap()
    wv_d = nc.dram_tensor("wv", [D, 256], F16, kind="ExternalInput").ap()
    wout_d = nc.dram_tensor("wout", [256, D], F16, kind="ExternalInput").ap()
    bqk_d = nc.dram_tensor("bqk", [128, 4], F32, kind="ExternalInput").ap()
    masks_d = nc.dram_tensor("masks", [128, 3, 2, 2, 128], F16,
                             kind="ExternalInput").ap()
    ident_d = nc.dram_tensor("ident", [128, 128], F16,
                             kind="ExternalInput").ap()
    out_d = nc.dram_tensor("out", [S, D], F16, kind="ExternalOutput").ap()

    with tile.TileContext(nc) as tc:
        with (
            tc.tile_pool(name="const", bufs=1) as cpool,
            tc.tile_pool(name="work", bufs=2) as wpool,
            tc.tile_pool(name="expp", bufs=10) as epool,
            tc.tile_pool(name="ysb", bufs=8) as ypool,
            tc.tile_pool(name="ps512", bufs=2, space="PSUM") as ps512,
            tc.tile_pool(name="psv", bufs=2, space="PSUM") as psv,
            tc.tile_pool(name="pssc", bufs=2, space="PSUM") as pssc,
            tc.tile_pool(name="psavT", bufs=2, space="PSUM") as psavT,
        ):
            # ---- persistent SBUF tensors ----
            xT_sb = cpool.tile([128, 8, S], F16, tag="xT")
            wqk_sb = cpool.tile([128, 8, 512], F16, tag="wqk")
            wv_sb = cpool.tile([128, 8, 256], F16, tag="wv")
            wout_sb = cpool.tile([128, 2, D], F16, tag="wout")
            bqk_sb = cpool.tile([128, 4], F32, tag="bqk")
            masks_sb = cpool.tile([128, 3, 2, 2, 128], F16, tag="masks")
            ident_sb = cpool.tile([128, 128], F16, tag="ident")
            qk_sb = cpool.tile([128, 4, S], F16, tag="qk")
            v_sb = cpool.tile([128, 19, 4, 65], F16, tag="v")
            aoT_sb = cpool.tile([128, 2, S], F16, tag="aoT")
            negc_sb = cpool.tile([128, 1], F32, tag="negc")

            # ---- input DMAs: ~256KB chunks in PE-consumption order, issue
            # cost (~650ns each) split across the two HWDGE queues (sync +
            # scalar) so transfers start early and overlap across rings ----
            xT_r = xT_d.rearrange("(ko kp) s -> kp ko s", kp=128)
            wqk_r = wqk_d.rearrange("(ko kp) n -> kp ko n", kp=128)
            wv_r = wv_d.rearrange("(ko kp) n -> kp ko n", kp=128)
            wout_r = wout_d.rearrange("(t p) n -> p t n", p=128)
            # Fine-grained DMAs on the sync queue in strict consumption
            # order: small chunks land with low latency and the issue stream
            # itself paces transfers so later chunks never steal bandwidth.
            # first 16 issues alternate sync/scalar: scalar is idle until
            # ~25us (qk drains live on DVE now), and two issue queues get
            # all 8 DMA rings transferring ~2.5us sooner
            for kt in range(8):
                eng_a = nc.sync if kt % 2 == 0 else nc.scalar
                eng_b = nc.scalar if kt % 2 == 0 else nc.sync
                eng_a.dma_start(out=wqk_sb[:, kt], in_=wqk_r[:, kt])
                eng_b.dma_start(out=xT_sb[:, kt, 0:512],
                                in_=xT_r[:, kt, 0:512])
            nc.sync.dma_start(out=bqk_sb[:], in_=bqk_d)
            nc.sync.dma_start(out=masks_sb[:], in_=masks_d)
            for kt in range(8):
                nc.sync.dma_start(out=xT_sb[:, kt, 512:1024],
                                  in_=xT_r[:, kt, 512:1024])
            for kt in range(8):
                nc.sync.dma_start(out=wv_sb[:, kt], in_=wv_r[:, kt])
            nc.sync.dma_start(out=ident_sb[:], in_=ident_d)
            nc.sync.dma_start(out=wout_sb[:], in_=wout_r[:])
            for kt in range(8):
                nc.sync.dma_start(out=xT_sb[:, kt, 1024:2048],
                                  in_=xT_r[:, kt, 1024:2048])

            # ---- constants (wsrc first so the PE warmup can start ASAP;
            # negc / v-ones deferred past the prologue so DVE drains the
            # first qk PSUM groups without queueing behind them) ----
            wsrc = cpool.tile([128, 512], F16, tag="wsrc")
            nc.vector.memset(wsrc[:, 0:128], 0.0)
            nc.vector.memset(wsrc[:, 128:512], 0.0)

            # ---- PE warmup: dummy matmuls on zeroed SBUF so the HAM
            # clock-gate ramps while the first input DMAs land ----
            wdst = cpool.tile([128, 512], F16, tag="wdst")
            wps = ps512.tile([128, 512], F32, tag="ps512")
            for w in range(N_WARMUP):
                nc.tensor.matmul(
                    out=wps[:],
                    lhsT=wsrc[:, 0:128],
                    rhs=wsrc[:],
                    start=(w == 0),
                    stop=(w == N_WARMUP - 1),
                )
            nc.scalar.copy(out=wdst[:], in_=wps[:])

            # ---- emission helpers ----
            def emit_qk_chunk(ns, pools=None, on_act=False):
                # all 4 m-tiles of q/k projection for seq chunk ns.
                # on_act: drain PSUM via ACT (idle in the prologue) instead
                # of DVE (busy mid-kernel).
                for m in range(4):
                    scale = SCALE if m < 2 else 1.0
                    if pools is None:
                        ps = ps512.tile([128, 512], F32, tag="ps512")
                    else:
                        pool, tg = pools[m % len(pools)]
                        ps = pool.tile([128, 512], F32, tag=tg)
                    for kt in range(8):
                        nc.tensor.matmul(
                            out=ps[:],
                            lhsT=wqk_sb[:, kt, m * 128:(m + 1) * 128],
                            rhs=xT_sb[:, kt, ns * 512:(ns + 1) * 512],
                            start=(kt == 0),
                            stop=(kt == 7),
                        )
                    if on_act:
                        nc.scalar.activation(
                            out=qk_sb[:, m, ns * 512:(ns + 1) * 512],
                            in_=ps[:],
                            func=mybir.ActivationFunctionType.Identity,
                            bias=bqk_sb[:, m:m + 1],
                            scale=scale,
                        )
                    else:
                        nc.vector.tensor_scalar(
                            out=qk_sb[:, m, ns * 512:(ns + 1) * 512],
                            in0=ps[:],
                            scalar1=scale,
                            scalar2=bqk_sb[:, m:m + 1],
                            op0=mybir.AluOpType.mult,
                            op1=mybir.AluOpType.add,
                        )

            def emit_qk_chunk_ktmajor(ns, pools):
                # kt-major: 4 open PSUM groups, consuming each xT k-chunk
                # as its DMA lands (used for the DMA-paced prologue chunks)
                pss = []
                for m in range(4):
                    pool, tg = pools[m % len(pools)]
                    ps = pool.tile([128, 512], F32, tag=tg, name=f"qkm{m}")
                    pss.append(ps)
                for kt in range(8):
                    for m in range(4):
                        nc.tensor.matmul(
                            out=pss[m][:],
                            lhsT=wqk_sb[:, kt, m * 128:(m + 1) * 128],
                            rhs=xT_sb[:, kt, ns * 512:(ns + 1) * 512],
                            start=(kt == 0),
                            stop=(kt == 7),
                        )
                for m in range(4):
                    nc.scalar.activation(
                        out=qk_sb[:, m, ns * 512:(ns + 1) * 512],
                        in_=pss[m][:],
                        func=mybir.ActivationFunctionType.Identity,
                        bias=bqk_sb[:, m:m + 1],
                        scale=SCALE if m < 2 else 1.0,
                    )

            def emit_v_chunk(c):
                off = OFFS[c]
                ps = psv.tile([128, 256], F32, tag="psv")
                for kt in range(8):
                    nc.tensor.matmul(
                        out=ps[:],
                        lhsT=xT_sb[:, kt, off:off + 128],
                        rhs=wv_sb[:, kt, :],
                        start=(kt == 0),
                        stop=(kt == 7),
                    )
                nc.scalar.copy(
                    out=v_sb[:, c, :, 0:64],
                    in_=ps[:].rearrange("p (h d) -> p h d", h=4),
                )

            def emit_v_pair(c):
                # two adjacent chunks through one PSUM bank + one ACT copy
                ps = psv.tile([128, 2, 256], F32, tag="psv", name="psvp")
                for j in range(2):
                    off = OFFS[c + j]
                    for kt in range(8):
                        nc.tensor.matmul(
                            out=ps[:, j, :],
                            lhsT=xT_sb[:, kt, off:off + 128],
                            rhs=wv_sb[:, kt, :],
                            start=(j == 0 and kt == 0),
                            stop=(j == 1 and kt == 7),
                        )
                nc.scalar.copy(
                    out=v_sb[:, c:c + 2, :, 0:64],
                    in_=ps[:].rearrange("p c (h d) -> p c h d", h=4),
                )

            def emit_scores(c4, hp):
                # scores + exp for both heads of the pair
                ex_big0 = epool.tile([128, 4, 2, 128], F16, tag="exp")
                ex_big1 = epool.tile([128, 4, 2, 128], F16, tag="exp")
                ex_big = {0: ex_big0, 1: ex_big1}
                for pair in range(2):
                    if c4 == 0 and pair == 0:
                        pv = 0
                    elif c4 == 3 and pair == 1:
                        pv = 2
                    else:
                        pv = 1
                    sc_h0 = pssc.tile([128, 2, 2, 128], F32, tag="pssc")
                    sc_h1 = pssc.tile([128, 2, 2, 128], F32, tag="pssc")
                    scs = {0: sc_h0, 1: sc_h1}
                    for iw in range(2):
                        ii = pair * 2 + iw
                        i = c4 * 4 + ii
                        cA, cB = _chunk_pair(i)
                        for hh in range(2):
                            po = hh * 64
                            for half, cc in enumerate((cA, cB)):
                                off = OFFS[cc]
                                nc.tensor.matmul(
                                    out=scs[hh][:, iw, half, :],
                                    lhsT=qk_sb[po:po + 64, 2 + hp,
                                               off:off + 128],
                                    rhs=qk_sb[po:po + 64, hp,
                                              i * 128:(i + 1) * 128],
                                    start=(iw == 0 and half == 0),
                                    stop=(iw == 1 and half == 1),
                                )
                    for hh in range(2):
                        # exp(score - C) on ACT (one op per ii-pair),
                        # band-zeroing via 0/1 mask multiply on DVE
                        sl = slice(pair * 2, pair * 2 + 2)
                        nc.scalar.activation(
                            out=ex_big[hh][:, sl],
                            in_=scs[hh][:],
                            func=mybir.ActivationFunctionType.Exp,
                            bias=negc_sb[:],
                        )
                        nc.vector.tensor_mul(
                            out=ex_big[hh][:, sl],
                            in0=ex_big[hh][:, sl],
                            in1=masks_sb[:, pv],
                        )
                return ex_big

            def emit_avT(c4, hp, ex_big):
                # AV^T: [q, dims+1] PSUM per (ii, head); ones column of v
                # yields the softmax denominator in col 64.
                avts = {}
                for hh in range(2):
                    h = 2 * hp + hh
                    avt = psavT.tile([128, 4, 65], F32, tag="psavT",
                                     padded_shape=[128, 4, 128])
                    for ii in range(4):
                        cA, cB = _chunk_pair(c4 * 4 + ii)
                        for half, cc in enumerate((cA, cB)):
                            nc.tensor.matmul(
                                out=avt[:, ii, :],
                                lhsT=ex_big[hh][:, ii, half, :],
                                rhs=v_sb[:, cc, h, 0:65],
                                start=(ii == 0 and half == 0),
                                stop=(ii == 3 and half == 1),
                            )
                    avts[hh] = avt
                # per-query reciprocal of the denominators, then normalize
                # with a stride-0 broadcast multiply (q is on partitions)
                recip = wpool.tile([128, 2, 4], F32, tag="recip")
                # ii-major layout so each ii slice is a contiguous [128, 128]
                # holding both heads — one PE transpose covers the pair
                avn = wpool.tile([128, 4, 2, 64], F16, tag="avn")
                for hh in range(2):
                    nc.vector.reciprocal(
                        out=recip[:, hh, :],
                        in_=avts[hh][:, :, 64:65].rearrange("p a b -> p (a b)"),
                    )
                    nc.vector.tensor_mul(
                        out=avn[:, :, hh, :],
                        in0=avts[hh][:, :, 0:64],
                        in1=_bcast_free(recip[:, hh, :], 64),
                    )
                return avn

            def emit_transposes(c4, hp, avn, split_copy=False):
                # PE-transpose normalized [q, 64] tiles into aoT [dims, seq]
                psT = psv.tile([128, 4, 128], F16, tag="psv")
                for ii in range(4):
                    nc.tensor.transpose(
                        out=psT[:, ii, :],
                        in_=avn[:, ii].rearrange("p a b -> p (a b)"),
                        identity=ident_sb[:],
                    )
                sl0 = c4 * 512
                if not split_copy:
                    # DVE: its queue clears sooner than ACT's (which is
                    # backed up behind the next block's exp ops)
                    nc.vector.tensor_copy(
                        out=aoT_sb[:, hp, sl0:sl0 + 512].rearrange(
                            "p (a b) -> p a b", a=4),
                        in_=psT[:],
                    )
                else:
                    # final block: per-ii copies, alternating ACT/DVE so
                    # the last outproj tiles' inputs land two at a time
                    for ii in range(4):
                        dst = aoT_sb[:, hp, sl0 + ii * 128:sl0 + ii * 128 + 128]
                        if ii % 2 == 0:
                            nc.scalar.copy(out=dst, in_=psT[:, ii, :])
                        else:
                            nc.vector.tensor_copy(out=dst, in_=psT[:, ii, :])

            def emit_outproj_st(st, pools=None, split_dma=False):
                ysb = ypool.tile([128, 1024], F16, tag="ysb")
                for nn in range(2):
                    if pools is None:
                        ps = ps512.tile([128, 512], F32, tag="ps512")
                    else:
                        pool, tg = pools[nn % len(pools)]
                        ps = pool.tile([128, 512], F32, tag=tg, name="pso")
                    for hp2 in range(2):
                        nc.tensor.matmul(
                            out=ps[:],
                            lhsT=aoT_sb[:, hp2, st * 128:(st + 1) * 128],
                            rhs=wout_sb[:, hp2,
                                        nn * 512:(nn + 1) * 512],
                            start=(hp2 == 0),
                            stop=(hp2 == 1),
                        )
                    if (st * 2 + nn) % 2 == 0:
                        nc.scalar.copy(out=ysb[:, nn * 512:(nn + 1) * 512],
                                       in_=ps[:])
                    else:
                        nc.vector.tensor_copy(
                            out=ysb[:, nn * 512:(nn + 1) * 512], in_=ps[:])
                if split_dma:
                    # final tiles: halve each transfer across two engines'
                    # rings so the drain tail shrinks (~37 GB/s per ring)
                    nc.sync.dma_start(
                        out=out_d[st * 128:(st + 1) * 128, 0:512],
                        in_=ysb[:, 0:512],
                    )
                    nc.gpsimd.dma_start(
                        out=out_d[st * 128:(st + 1) * 128, 512:1024],
                        in_=ysb[:, 512:1024],
                    )
                else:
                    nc.sync.dma_start(
                        out=out_d[st * 128:(st + 1) * 128, :],
                        in_=ysb[:],
                    )

            # ---- emission schedule ----
            # prologue: q/k for seq 0:1024, v chunks for the c4=0 blocks
            rrp = [(ps512, "ps512"), (pssc, "pssc"),
                   (ps512, "ps512"), (pssc, "pssc")]
            emit_qk_chunk(0, pools=rrp)
            emit_qk_chunk(1, pools=rrp)
            nc.vector.memset(negc_sb[:], -C_SUB)
            nc.vector.memset(v_sb[:, :, :, 64:65], 1.0)
            for c in (15, 0, 1, 2, 3):
                emit_v_chunk(c)

            # per-block fillers: A runs between scores and AV^T (covers the
            # exp/mask latency AND the previous block's norm latency via its
            # transposes), B runs after AV^T (covers this block's norm).
            fillerA = {
                (0, 0): [("v", 4), ("v", 5), ("v", 6), ("v", 7)],
                (0, 1): [("qk", 2)],
                (1, 0): [("qk", 3), ("T", (0, 0))],
                (1, 1): [("T", (0, 1))],
                (2, 0): [("st", 0), ("st", 1), ("T", (1, 0))],
                (2, 1): [("st", 2), ("st", 3), ("T", (1, 1))],
                (3, 0): [("st", 4), ("st", 5), ("T", (2, 0))],
                (3, 1): [("st", 6), ("st", 7), ("T", (2, 1))],
            }
            fillerB = {
                (0, 0): [],
                (0, 1): [("v", 8), ("v", 9)],
                (1, 0): [("v", 10), ("v", 11)],
                (1, 1): [],
                (2, 0): [("v", 12), ("v", 13)],
                (2, 1): [("v", 14), ("v", 18)],
                (3, 0): [],
                (3, 1): [("st", 8), ("st", 9)],
            }
            avns = {}
            # psavT slots are drained (normed) by the time fillerA runs, so
            # outproj tiles can rotate through them alongside ps512
            trr_mid = [(ps512, "ps512"), (psavT, "psavT")]

            def run_items(items):
                for kind, it in items:
                    if kind == "qk":
                        emit_qk_chunk(it)
                    elif kind == "v":
                        emit_v_chunk(it)
                    elif kind == "vp":
                        emit_v_pair(it)
                    elif kind == "st":
                        emit_outproj_st(it, pools=trr_mid)
                    else:
                        emit_transposes(*it, avns.pop(it))

            for c4 in range(4):
                for hp in range(2):
                    ex_big = emit_scores(c4, hp)
                    run_items(fillerA[(c4, hp)])
                    avns[(c4, hp)] = emit_avT(c4, hp, ex_big)
                    run_items(fillerB[(c4, hp)])

            # tail: last two transpose groups + remaining outproj tiles.
            # psavT is drained by now — rotate its banks in so four outproj
            # PSUM groups stay open against the ACT/DVE copy latency.
            trr = [(ps512, "ps512"), (psavT, "psavT")]
            emit_transposes(3, 0, avns.pop((3, 0)))
            emit_outproj_st(10, pools=trr)
            emit_outproj_st(11, pools=trr)
            emit_transposes(3, 1, avns.pop((3, 1)), split_copy=True)
            for st in range(12, 16):
                emit_outproj_st(st, pools=trr)

    nc.compile()
    return nc


_NC = None


def _get_program():
    global _NC
    if _NC is None:
        _NC = _build_program()
    return _NC


def _make_in_maps(x, w_qkv, b_qkv, w_out):
    masks = _build_pair_masks()
    ident = np.eye(128, dtype=np.float16)

    in_maps = []
    for c in range(8):
        b, hg = divmod(c, 4)
        cq = 256 * hg
        wqk = np.concatenate(
            [w_qkv[:, cq:cq + 256], w_qkv[:, 1024 + cq:1024 + cq + 256]],
            axis=1,
        ).astype(np.float16)
        bqk = np.empty((128, 4), np.float32)
        bqk[:, 0] = b_qkv[cq:cq + 128] * SCALE
        bqk[:, 1] = b_qkv[cq + 128:cq + 256] * SCALE
        bqk[:, 2] = b_qkv[1024 + cq:1024 + cq + 128]
        bqk[:, 3] = b_qkv[1024 + cq + 128:1024 + cq + 256]
        in_maps.append({
            "xT": np.ascontiguousarray(x[b].T).astype(np.float16),
            "wqk": wqk,
            "wv": w_qkv[:, 2048 + cq:2048 + cq + 256].astype(np.float16),
            "wout": w_out[cq:cq + 256, :].astype(np.float16),
            "bqk": bqk,
            "masks": masks,
            "ident": ident,
        })
    return in_maps


def kernel(x, w_qkv, b_qkv, w_out, b_out):
    x = np.asarray(x, np.float32)
    w_qkv = np.asarray(w_qkv, np.float32)
    b_qkv = np.asarray(b_qkv, np.float32)
    w_out = np.asarray(w_out, np.float32)
    b_out = np.asarray(b_out, np.float32)

    in_maps = _make_in_maps(x, w_qkv, b_qkv, w_out)
    nc = _get_program()
    res = run_bass_kernel_spmd(nc, in_maps, list(range(8)))

    b_v = b_qkv[2048:]
    bias_all = b_out + b_v @ w_out  # folds the (untracked) v-bias
    y = np.empty((B, S, D), np.float32)
    for b in range(B):
        acc = np.zeros((S, D), np.float32)
        for hg in range(4):
            acc += res.results[4 * b + hg]["out"].astype(np.float32)
        y[b] = acc + bias_all
    return y


# revision 84
# speedup vs baseline: 1.1715x; 1.0017x over previous
"""Trainium2 Bass kernel for LocalSparseAttention.

Problem (hardcoded): B=2, S=2048, D=1024, H=16, HD=64, WINDOW=128 (band
|i-j| <= 64), fp32 I/O.

Sharding: 8 cores = 2 batches x 4 head-groups (4 heads each). Each core:
  - qk projection into transposed layout [512, 2048] (head-pair packed)
  - v projection into natural layout, 17 (possibly 64-shifted) seq chunks
    (boundary tiles reuse shifted chunks; masks dedup the overlap)
  - banded attention per 128-query tile with a 256-key window:
      scores  -> exp on ACT -> 0/1 band-mask multiply on DVE
      AV^T    : lhsT=exp tile [keys, q], rhs=v (+ ones col) -> PSUM [q, 65]
                (full 128x128 PE efficiency; denominator = ones column)
      norm    : per-partition reciprocal + stride-0-broadcast multiply
                (DVE for even head, GpSimd for odd head)
      PE transpose of normalized [q, 64] tiles back into aoT [dims, seq]
  - output projection -> fp16 partial [2048, 1024]
Host: fp16 casts/transposes in, sum of 4 partials per batch + fused bias
(b_out + b_v @ w_out) out.

All matmuls run in fp16 (1 cycle/row on PE) with fp32 PSUM accumulation;
softmax exp input stays fp32.
"""
import sys

if "/opt/trn_rl_repo" not in sys.path:
    sys.path.insert(0, "/opt/trn_rl_repo")

import numpy as np

import concourse.bass as bass
import concourse.mybir as mybir
import concourse.tile as tile
from concourse import bacc
from concourse.bass import AP
from concourse.bass_utils import run_bass_kernel_spmd

B, S, D, H, HD = 2, 2048, 1024, 16, 64
SCALE = HD**-0.5
C_SUB = 4.0  # subtracted from all scores via the exp bias; cancels in softmax

F16 = mybir.dt.float16
F32 = mybir.dt.float32

# 19 key/value chunk offsets: 15 shifted (128c+64) + aligned 0,128,1792,1920
OFFS = [128 * c + 64 for c in range(15)] + [0, 128, 1792, 1920]

N_WARMUP = 22


def _chunk_pair(i):
    # Boundary tiles reuse shifted chunks (the mask dedups the overlap),
    # so only 17 v chunks are ever materialized (15 shifted + 0 + 1920).
    if i == 0:
        return 15, 0
    if i == 15:
        return 14, 18
    return i - 1, i


def _half_mask(q_base, k_base, own_lo, own_hi):
    kp = np.arange(128)[:, None]
    q = np.arange(128)[None, :]
    k = k_base + kp
    valid = (k >= own_lo) & (k < own_hi) & (np.abs((q_base + q) - k) <= 64)
    return valid.astype(np.float16)


def _build_pair_masks():
    # variant 0: (tile 0, interior) — c4=0 pair 0
    # variant 1: (interior, interior)
    # variant 2: (interior, tile 15) — c4=3 pair 1
    # Each [kp, q] mask owns a disjoint global-key range so overlapping
    # chunk halves never double-count a key.
    mp = np.zeros((128, 3, 2, 2, 128), np.float16)
    int0 = _half_mask(128, 64, 64, 192)       # any interior tile, half 0
    int1 = _half_mask(128, 192, 192, 320)     # any interior tile, half 1
    mp[:, 1, :, 0] = int0[:, None]
    mp[:, 1, :, 1] = int1[:, None]
    mp[:, 0] = mp[:, 2] = mp[:, 1]
    mp[:, 0, 0, 0] = _half_mask(0, 0, 0, 128)      # tile 0 vs chunk 15
    mp[:, 0, 0, 1] = _half_mask(0, 64, 128, 192)   # tile 0 vs chunk 0
    mp[:, 2, 1, 0] = _half_mask(1920, 1856, 1856, 1984)  # tile 15 / chunk 14
    mp[:, 2, 1, 1] = _half_mask(1920, 1920, 1984, 2048)  # tile 15 / chunk 18
    return mp


def _bcast_free(ap, n):
    # append a stride-0 free dim of size n (broadcast along free axis)
    return AP(ap.tensor, ap.offset, list(ap.ap) + [[0, n]])


def _build_program():
    nc = bacc.Bacc("TRN2", debug=False, num_devices=8)

    xT_d = nc.dram_tensor("xT", [D, S], F16, kind="ExternalInput").ap()
    wqk_d = nc.dram_tensor("wqk", [D, 512], F16, kind="ExternalInput").ap()
    wv_d = nc.dram_tensor("wv", [D, 256], F16, kind="ExternalInput").ap()
    wout_d = nc.dram_tensor("wout", [256, D], F16, kind="ExternalInput").ap()
    bqk_d = nc.dram_tensor("bqk", [128, 4], F32, kind="ExternalInput").ap()
    masks_d = nc.dram_tensor("masks", [128, 3, 2, 2, 128], F16,
                             kind="ExternalInput").ap()
    ident_d = nc.dram_tensor("ident", [128, 128], F16,
                             kind="ExternalInput").ap()
    out_d = nc.dram_tensor("out", [S, D], F16, kind="ExternalOutput").ap()

    with tile.TileContext(nc) as tc:
        with (
            tc.tile_pool(name="const", bufs=1) as cpool,
            tc.tile_pool(name="work", bufs=2) as wpool,
            tc.tile_pool(name="expp", bufs=10) as epool,
            tc.tile_pool(name="ysb", bufs=8) as ypool,
            tc.tile_pool(name="ps512", bufs=2, space="PSUM") as ps512,
            tc.tile_pool(name="psv", bufs=2, space="PSUM") as psv,
            tc.tile_pool(name="pssc", bufs=2, space="PSUM") as pssc,
            tc.tile_pool(name="psavT", bufs=2, space="PSUM") as psavT,
        ):
            # ---- persistent SBUF tensors ----
            xT_sb = cpool.tile([128, 8, S], F16, tag="xT")
            wqk_sb = cpool.tile([128, 8, 512], F16, tag="wqk")
            wv_sb = cpool.tile([128, 8, 256], F16, tag="wv")
            wout_sb = cpool.tile([128, 2, D], F16, tag="wout")
            bqk_sb = cpool.tile([128, 4], F32, tag="bqk")
            masks_sb = cpool.tile([128, 3, 2, 2, 128], F16, tag="masks")
            ident_sb = cpool.tile([128, 128], F16, tag="ident")
            qk_sb = cpool.tile([128, 4, S], F16, tag="qk")
            v_sb = cpool.tile([128, 19, 4, 65], F16, tag="v")
            aoT_sb = cpool.tile([128, 2, S], F16, tag="aoT")
            negc_sb = cpool.tile([128, 1], F32, tag="negc")

            # ---- input DMAs: ~256KB chunks in PE-consumption order, issue
            # cost (~650ns each) split across the two HWDGE queues (sync +
            # scalar) so transfers start early and overlap across rings ----
            xT_r = xT_d.rearrange("(ko kp) s -> kp ko s", kp=128)
            wqk_r = wqk_d.rearrange("(ko kp) n -> kp ko n", kp=128)
            wv_r = wv_d.rearrange("(ko kp) n -> kp ko n", kp=128)
            wout_r = wout_d.rearrange("(t p) n -> p t n", p=128)
            # Fine-grained DMAs on the sync queue in strict consumption
            # order: small chunks land with low latency and the issue stream
            # itself paces transfers so later chunks never steal bandwidth.
            # first 16 issues alternate sync/scalar: scalar is idle until
            # ~25us (qk drains live on DVE now), and two issue queues get
            # all 8 DMA rings transferring ~2.5us sooner
            for kt in range(8):
                eng_a = nc.sync if kt % 2 == 0 else nc.scalar
                eng_b = nc.scalar if kt % 2 == 0 else nc.sync
                eng_a.dma_start(out=wqk_sb[:, kt], in_=wqk_r[:, kt])
                eng_b.dma_start(out=xT_sb[:, kt, 0:512],
                                in_=xT_r[:, kt, 0:512])
            nc.sync.dma_start(out=bqk_sb[:], in_=bqk_d)
            nc.sync.dma_start(out=masks_sb[:], in_=masks_d)
            for kt in range(8):
                nc.sync.dma_start(out=xT_sb[:, kt, 512:1024],
                                  in_=xT_r[:, kt, 512:1024])
            for kt in range(8):
                nc.sync.dma_start(out=wv_sb[:, kt], in_=wv_r[:, kt])
            nc.sync.dma_start(out=ident_sb[:], in_=ident_d)
            nc.sync.dma_start(out=wout_sb[:], in_=wout_r[:])
            for kt in range(8):
                nc.sync.dma_start(out=xT_sb[:, kt, 1024:2048],
                                  in_=xT_r[:, kt, 1024:2048])

            # ---- constants (wsrc first so the PE warmup can start ASAP;
            # negc / v-ones deferred past the prologue so DVE drains the
            # first qk PSUM groups without queueing behind them) ----
            wsrc = cpool.tile([128, 512], F16, tag="wsrc")
            nc.vector.memset(wsrc[:, 0:128], 0.0)
            nc.vector.memset(wsrc[:, 128:512], 0.0)

            # ---- PE warmup: dummy matmuls on zeroed SBUF so the HAM
            # clock-gate ramps while the first input DMAs land ----
            wdst = cpool.tile([128, 512], F16, tag="wdst")
            wps = ps512.tile([128, 512], F32, tag="ps512")
            for w in range(N_WARMUP):
                nc.tensor.matmul(
                    out=wps[:],
                    lhsT=wsrc[:, 0:128],
                    rhs=wsrc[:],
                    start=(w == 0),
                    stop=(w == N_WARMUP - 1),
                )
            nc.scalar.copy(out=wdst[:], in_=wps[:])

            # ---- emission helpers ----
            def emit_qk_chunk(ns, pools=None, on_act=False):
                # all 4 m-tiles of q/k projection for seq chunk ns.
                # on_act: drain PSUM via ACT (idle in the prologue) instead
                # of DVE (busy mid-kernel).
                for m in range(4):
                    scale = SCALE if m < 2 else 1.0
                    if pools is None:
                        ps = ps512.tile([128, 512], F32, tag="ps512")
                    else:
                        pool, tg = pools[m % len(pools)]
                        ps = pool.tile([128, 512], F32, tag=tg)
                    for kt in range(8):
                        nc.tensor.matmul(
                            out=ps[:],
                            lhsT=wqk_sb[:, kt, m * 128:(m + 1) * 128],
                            rhs=xT_sb[:, kt, ns * 512:(ns + 1) * 512],
                            start=(kt == 0),
                            stop=(kt == 7),
                        )
                    if on_act:
                        nc.scalar.activation(
                            out=qk_sb[:, m, ns * 512:(ns + 1) * 512],
                            in_=ps[:],
                            func=mybir.ActivationFunctionType.Identity,
                            bias=bqk_sb[:, m:m + 1],
                            scale=scale,
                        )
                    else:
                        nc.vector.tensor_scalar(
                            out=qk_sb[:, m, ns * 512:(ns + 1) * 512],
                            in0=ps[:],
                            scalar1=scale,
                            scalar2=bqk_sb[:, m:m + 1],
                            op0=mybir.AluOpType.mult,
                            op1=mybir.AluOpType.add,
                        )

            def emit_qk_chunk_ktmajor(ns, pools):
                # kt-major: 4 open PSUM groups, consuming each xT k-chunk
                # as its DMA lands (used for the DMA-paced prologue chunks)
                pss = []
                for m in range(4):
                    pool, tg = pools[m % len(pools)]
                    ps = pool.tile([128, 512], F32, tag=tg, name=f"qkm{m}")
                    pss.append(ps)
                for kt in range(8):
                    for m in range(4):
                        nc.tensor.matmul(
                            out=pss[m][:],
                            lhsT=wqk_sb[:, kt, m * 128:(m + 1) * 128],
                            rhs=xT_sb[:, kt, ns * 512:(ns + 1) * 512],
                            start=(kt == 0),
                            stop=(kt == 7),
                        )
                for m in range(4):
                    nc.scalar.activation(
                        out=qk_sb[:, m, ns * 512:(ns + 1) * 512],
                        in_=pss[m][:],
                        func=mybir.ActivationFunctionType.Identity,
                        bias=bqk_sb[:, m:m + 1],
                        scale=SCALE if m < 2 else 1.0,
                    )

            def emit_v_chunk(c):
                off = OFFS[c]
                ps = psv.tile([128, 256], F32, tag="psv")
                for kt in range(8):
                    nc.tensor.matmul(
                        out=ps[:],
                        lhsT=xT_sb[:, kt, off:off + 128],
                        rhs=wv_sb[:, kt, :],
                        start=(kt == 0),
                        stop=(kt == 7),
                    )
                nc.scalar.copy(
                    out=v_sb[:, c, :, 0:64],
                    in_=ps[:].rearrange("p (h d) -> p h d", h=4),
                )

            def emit_v_pair(c):
                # two adjacent chunks through one PSUM bank + one ACT copy
                ps = psv.tile([128, 2, 256], F32, tag="psv", name="psvp")
                for j in range(2):
                    off = OFFS[c + j]
                    for kt in range(8):
                        nc.tensor.matmul(
                            out=ps[:, j, :],
                            lhsT=xT_sb[:, kt, off:off + 128],
                            rhs=wv_sb[:, kt, :],
                            start=(j == 0 and kt == 0),
                            stop=(j == 1 and kt == 7),
                        )
                nc.scalar.copy(
                    out=v_sb[:, c:c + 2, :, 0:64],
                    in_=ps[:].rearrange("p c (h d) -> p c h d", h=4),
                )

            def emit_scores(c4, hp):
                # scores + exp for both heads of the pair
                ex_big0 = epool.tile([128, 4, 2, 128], F16, tag="exp")
                ex_big1 = epool.tile([128, 4, 2, 128], F16, tag="exp")
                ex_big = {0: ex_big0, 1: ex_big1}
                for pair in range(2):
                    if c4 == 0 and pair == 0:
                        pv = 0
                    elif c4 == 3 and pair == 1:
                        pv = 2
                    else:
                        pv = 1
                    sc_h0 = pssc.tile([128, 2, 2, 128], F32, tag="pssc")
                    sc_h1 = pssc.tile([128, 2, 2, 128], F32, tag="pssc")
                    scs = {0: sc_h0, 1: sc_h1}
                    for iw in range(2):
                        ii = pair * 2 + iw
                        i = c4 * 4 + ii
                        cA, cB = _chunk_pair(i)
                        for hh in range(2):
                            po = hh * 64
                            for half, cc in enumerate((cA, cB)):
                                off = OFFS[cc]
                                nc.tensor.matmul(
                                    out=scs[hh][:, iw, half, :],
                                    lhsT=qk_sb[po:po + 64, 2 + hp,
                                               off:off + 128],
                                    rhs=qk_sb[po:po + 64, hp,
                                              i * 128:(i + 1) * 128],
                                    start=(iw == 0 and half == 0),
                                    stop=(iw == 1 and half == 1),
                                )
                    for hh in range(2):
                        # exp(score - C) on ACT (one op per ii-pair),
                        # band-zeroing via 0/1 mask multiply on DVE
                        sl = slice(pair * 2, pair * 2 + 2)
                        nc.scalar.activation(
                            out=ex_big[hh][:, sl],
                            in_=scs[hh][:],
                            func=mybir.ActivationFunctionType.Exp,
                            bias=negc_sb[:],
                        )
                        nc.vector.tensor_mul(
                            out=ex_big[hh][:, sl],
                            in0=ex_big[hh][:, sl],
                            in1=masks_sb[:, pv],
                        )
                return ex_big

            def emit_avT(c4, hp, ex_big):
                # AV^T: [q, dims+1] PSUM per (ii, head); ones column of v
                # yields the softmax denominator in col 64.
                avts = {}
                for hh in range(2):
                    h = 2 * hp + hh
                    avt = psavT.tile([128, 4, 65], F32, tag="psavT",
                                     padded_shape=[128, 4, 128])
                    for ii in range(4):
                        cA, cB = _chunk_pair(c4 * 4 + ii)
                        for half, cc in enumerate((cA, cB)):
                            nc.tensor.matmul(
                                out=avt[:, ii, :],
                                lhsT=ex_big[hh][:, ii, half, :],
                                rhs=v_sb[:, cc, h, 0:65],
                                start=(ii == 0 and half == 0),
                                stop=(ii == 3 and half == 1),
                            )
                    avts[hh] = avt
                # per-query reciprocal of the denominators, then normalize
                # with a stride-0 broadcast multiply (q is on partitions)
                recip = wpool.tile([128, 2, 4], F32, tag="recip")
                # ii-major layout so each ii slice is a contiguous [128, 128]
                # holding both heads — one PE transpose covers the pair
                avn = wpool.tile([128, 4, 2, 64], F16, tag="avn")
                for hh in range(2):
                    nc.vector.reciprocal(
                        out=recip[:, hh, :],
                        in_=avts[hh][:, :, 64:65].rearrange("p a b -> p (a b)"),
                    )
                    nc.vector.tensor_mul(
                        out=avn[:, :, hh, :],
                        in0=avts[hh][:, :, 0:64],
                        in1=_bcast_free(recip[:, hh, :], 64),
                    )
                return avn

            def emit_transposes(c4, hp, avn, split_copy=False):
                # PE-transpose normalized [q, 64] tiles into aoT [dims, seq]
                psT = psv.tile([128, 4, 128], F16, tag="psv")
                for ii in range(4):
                    nc.tensor.transpose(
                        out=psT[:, ii, :],
                        in_=avn[:, ii].rearrange("p a b -> p (a b)"),
                        identity=ident_sb[:],
                    )
                sl0 = c4 * 512
                if not split_copy:
                    # DVE: its queue clears sooner than ACT's (which is
                    # backed up behind the next block's exp ops)
                    nc.vector.tensor_copy(
                        out=aoT_sb[:, hp, sl0:sl0 + 512].rearrange(
                            "p (a b) -> p a b", a=4),
                        in_=psT[:],
                    )
                else:
                    # final block: per-ii copies, alternating ACT/DVE so
                    # the last outproj tiles' inputs land two at a time
                    for ii in range(4):
                        dst = aoT_sb[:, hp, sl0 + ii * 128:sl0 + ii * 128 + 128]
                        if ii % 2 == 0:
                            nc.scalar.copy(out=dst, in_=psT[:, ii, :])
                        else:
                            nc.vector.tensor_copy(out=dst, in_=psT[:, ii, :])

            def emit_outproj_st(st, pools=None, split_dma=False):
                ysb = ypool.tile([128, 1024], F16, tag="ysb")
                for nn in range(2):
                    if pools is None:
                        ps = ps512.tile([128, 512], F32, tag="ps512")
                    else:
                        pool, tg = pools[nn % len(pools)]
                        ps = pool.tile([128, 512], F32, tag=tg, name="pso")
                    for hp2 in range(2):
                        nc.tensor.matmul(
                            out=ps[:],
                            lhsT=aoT_sb[:, hp2, st * 128:(st + 1) * 128],
                            rhs=wout_sb[:, hp2,
                                        nn * 512:(nn + 1) * 512],
                            start=(hp2 == 0),
                            stop=(hp2 == 1),
                        )
                    if (st * 2 + nn) % 2 == 0:
                        nc.scalar.copy(out=ysb[:, nn * 512:(nn + 1) * 512],
                                       in_=ps[:])
                    else:
                        nc.vector.tensor_copy(
                            out=ysb[:, nn * 512:(nn + 1) * 512], in_=ps[:])
                if split_dma:
                    # final tiles: halve each transfer across two engines'
                    # rings so the drain tail shrinks (~37 GB/s per ring)
                    nc.sync.dma_start(
                        out=out_d[st * 128:(st + 1) * 128, 0:512],
                        in_=ysb[:, 0:512],
                    )
                    nc.gpsimd.dma_start(
                        out=out_d[st * 128:(st + 1) * 128, 512:1024],
                        in_=ysb[:, 512:1024],
                    )
                else:
                    nc.sync.dma_start(
                        out=out_d[st * 128:(st + 1) * 128, :],
                        in_=ysb[:],
                    )

            # ---- emission schedule ----
            # prologue: q/k for seq 0:1024, v chunks for the c4=0 blocks
            rrp = [(ps512, "ps512"), (pssc, "pssc"),
                   (ps512, "ps512"), (pssc, "pssc")]
            emit_qk_chunk(0, pools=rrp)
            emit_qk_chunk(1, pools=rrp)
            nc.vector.memset(negc_sb[:], -C_SUB)
            nc.vector.memset(v_sb[:, :, :, 64:65], 1.0)
            for c in (15, 0, 1, 2, 3):
                emit_v_chunk(c)

            # per-block fillers: A runs between scores and AV^T (covers the
            # exp/mask latency AND the previous block's norm latency via its
            # transposes), B runs after AV^T (covers this block's norm).
            fillerA = {
                (0, 0): [("v", 4), ("v", 5), ("v", 6), ("v", 7)],
                (0, 1): [("qk", 2)],
                (1, 0): [("qk", 3), ("T", (0, 0))],
                (1, 1): [("T", (0, 1))],
                (2, 0): [("st", 0), ("st", 1), ("T", (1, 0))],
                (2, 1): [("st", 2), ("st", 3), ("T", (1, 1))],
                (3, 0): [("st", 4), ("st", 5), ("T", (2, 0))],
                (3, 1): [("st", 6), ("st", 7), ("T", (2, 1))],
            }
            fillerB = {
                (0, 0): [],
                (0, 1): [("v", 8), ("v", 9)],
                (1, 0): [("v", 10), ("v", 11)],
                (1, 1): [],
                (2, 0): [("v", 12), ("v", 13)],
                (2, 1): [("v", 14), ("v", 18)],
                (3, 0): [],
                (3, 1): [("st", 8), ("st", 9)],
            }
            avns = {}
            # psavT slots are drained (normed) by the time fillerA runs, so
            # outproj tiles can rotate through them alongside ps512
            trr_mid = [(ps512, "ps512"), (psavT, "psavT")]

            def run_items(items):
                for kind, it in items:
                    if kind == "qk":
                        emit_qk_chunk(it)
                    elif kind == "v":
                        emit_v_chunk(it)
                    elif kind == "vp":
                        emit_v_pair(it)
                    elif kind == "st":
                        emit_outproj_st(it, pools=trr_mid)
                    else:
                        emit_transposes(*it, avns.pop(it))

            for c4 in range(4):
                for hp in range(2):
                    ex_big = emit_scores(c4, hp)
                    run_items(fillerA[(c4, hp)])
                    avns[(c4, hp)] = emit_avT(c4, hp, ex_big)
                    run_items(fillerB[(c4, hp)])

            # tail: last two transpose groups + remaining outproj tiles.
            # psavT is drained by now — rotate its banks in so four outproj
            # PSUM groups stay open against the ACT/DVE copy latency.
            trr = [(ps512, "ps512"), (psavT, "psavT")]
            emit_transposes(3, 0, avns.pop((3, 0)))
            emit_outproj_st(10, pools=trr)
            emit_outproj_st(11, pools=trr)
            emit_transposes(3, 1, avns.pop((3, 1)), split_copy=True)
            for st in range(12, 16):
                emit_outproj_st(st, pools=trr)

    nc.compile()
    return nc


_NC = None


def _get_program():
    global _NC
    if _NC is None:
        _NC = _build_program()
    return _NC


def _make_in_maps(x, w_qkv, b_qkv, w_out):
    masks = _build_pair_masks()
    ident = np.eye(128, dtype=np.float16)

    in_maps = []
    for c in range(8):
        b, hg = divmod(c, 4)
        cq = 256 * hg
        wqk = np.concatenate(
            [w_qkv[:, cq:cq + 256], w_qkv[:, 1024 + cq:1024 + cq + 256]],
            axis=1,
        ).astype(np.float16)
        bqk = np.empty((128, 4), np.float32)
        bqk[:, 0] = b_qkv[cq:cq + 128] * SCALE
        bqk[:, 1] = b_qkv[cq + 128:cq + 256] * SCALE
        bqk[:, 2] = b_qkv[1024 + cq:1024 + cq + 128]
        bqk[:, 3] = b_qkv[1024 + cq + 128:1024 + cq + 256]
        in_maps.append({
            "xT": np.ascontiguousarray(x[b].T).astype(np.float16),
            "wqk": wqk,
            "wv": w_qkv[:, 2048 + cq:2048 + cq + 256].astype(np.float16),
            "wout": w_out[cq:cq + 256, :].astype(np.float16),
            "bqk": bqk,
            "masks": masks,
            "ident": ident,
        })
    return in_maps


def kernel(x, w_qkv, b_qkv, w_out, b_out):
    x = np.asarray(x, np.float32)
    w_qkv = np.asarray(w_qkv, np.float32)
    b_qkv = np.asarray(b_qkv, np.float32)
    w_out = np.asarray(w_out, np.float32)
    b_out = np.asarray(b_out, np.float32)

    in_maps = _make_in_maps(x, w_qkv, b_qkv, w_out)
    nc = _get_program()
    res = run_bass_kernel_spmd(nc, in_maps, list(range(8)))

    b_v = b_qkv[2048:]
    bias_all = b_out + b_v @ w_out  # folds the (untracked) v-bias
    y = np.empty((B, S, D), np.float32)
    for b in range(B):
        acc = np.zeros((S, D), np.float32)
        for hg in range(4):
            acc += res.results[4 * b + hg]["out"].astype(np.float32)
        y[b] = acc + bias_all
    return y
